# revision 1
# baseline (speedup 1.0000x reference)
"""Multi-head attention (RoPE, causal) Trainium2 Bass kernel, 8 NeuronCores.

Problem: x[4,2048,1024] -> MHA(16 heads, head_dim 64, RoPE, causal mask) -> [4,2048,1024]

Sharding (pure data/tensor parallel, no collectives):
  core c -> (batch b = c//2, head-group g = c%2); each head-group = 8 heads = 512 dims.
  Each core computes q/k/v projections for its (batch, head-group), RoPE, attention,
  and a partial output projection (columns of Wo for its head group).
  Host sums the two partial outputs per batch (512-dim contraction split).

Kernel layout tricks:
  - Projections computed in transposed [out_dim, seq] layout (QT/KT) so that
    QK^T blocks come out as S^T [k, q]: softmax reductions along the partition
    dim are avoided entirely via UNSAFE softmax (no row-max; inputs are bounded
    N(0,1)-ish data, logits stay << 88) and the row-sum is folded into the PV
    matmul by augmenting V with a ones column.  No on-chip transposes anywhere.
  - V is computed in natural [seq, dim] layout (lhsT = xT chunks) for PV.
  - RoPE via a signed-permutation matrix on the TensorEngine (rot = Psig @ pre)
    plus 3 VectorEngine elementwise ops per chunk; the final add writes bf16
    Q/K tiles directly.
  - Projections / out-proj in float32r (full-rate fp32, moving dim >= 256);
    attention QK'/PV in bf16 (fast weight load, cheap LDWEIGHTS).
  - Causal masking: lower blocks computed unmasked, diagonal-strip blocks get a
    0/1 bf16 mask multiply; upper blocks skipped entirely.
  - Softmax normalization: 1/l batched on 4-head tiles (partitions 0/32/64/96),
    PE-broadcast of 1/l over each head's 64 rows, fused multiply into aT.
"""

import numpy as np
import ml_dtypes

import concourse.bass as bass
import concourse.tile as tile
from concourse import bacc, mybir
from concourse import bass_utils

B, S, D, H, DH = 4, 2048, 1024, 16, 64
NCORES = 8
HG = 2              # head groups (tensor parallel)
HPG = H // HG       # heads per group = 8
OG = HPG * DH       # group output dims = 512
SCALE = DH ** -0.5
P = 128
QSB = 512           # q super-block width
NQSB = S // QSB     # 4
KB = 128            # k block
NKB = S // KB       # 16
DC = D // P         # 8 d-chunks
JC = OG // P        # 4 j-chunks (out-proj contraction)

F32 = mybir.dt.float32
F32R = mybir.dt.float32r
BF16 = mybir.dt.bfloat16

_COMPILED = {}


# ---------------------------------------------------------------- host tables

def _rope_tables():
    inv_freq = 1.0 / (10000.0 ** (np.arange(0, DH, 2, dtype=np.float32) / DH))
    t = np.arange(S, dtype=np.float32)
    freqs = np.outer(t, inv_freq).astype(np.float32)      # [S, 32]
    emb = np.concatenate([freqs, freqs], -1)              # [S, 64]
    return np.cos(emb), np.sin(emb)


def _host_consts():
    cos, sin = _rope_tables()                             # [S, 64]
    cosT2 = np.ascontiguousarray(
        np.concatenate([cos.T, cos.T], axis=0), dtype=np.float32)   # [128, S]
    sinT2 = np.ascontiguousarray(
        np.concatenate([sin.T, sin.T], axis=0), dtype=np.float32)
    # signed permutation: rot(x)[i] = -x[i+32] (j<32) else x[i-32], per 64-row head
    psig = np.zeros((P, P), np.float32)
    for i in range(P):
        j = i % DH
        base = (i // DH) * DH
        if j < 32:
            psig[i, base + j + 32] = -1.0
        else:
            psig[i, base + j - 32] = 1.0
    psigT = np.ascontiguousarray(psig.T)
    return cosT2, sinT2, psigT


def _mask_plan(mask):
    """Classify the [S, S] mask into a per-qsb block plan.

    plan[qsb] = list of (kb, msel); msel is None (no mask), ("const", r) for
    the 4 shared causal diagonal tiles, or ("dram", qsb, kb) for generic
    per-block mask tiles.
    """
    m = np.asarray(mask).reshape(S, S) != 0        # [q, k] True = attend
    causal = np.array_equal(m, np.tril(np.ones((S, S), bool)))
    if causal:
        plan = []
        for qsb in range(NQSB):
            row = []
            for kb in range(4 * qsb + 4):
                r = kb - 4 * qsb
                row.append((kb, None if r < 0 else ("const", r)))
            plan.append(row)
        return plan, "causal"
    if m.all():
        return [[(kb, None) for kb in range(NKB)] for _ in range(NQSB)], "full"
    plan = []
    for qsb in range(NQSB):
        row = []
        for kb in range(NKB):
            blk = m[qsb * QSB:(qsb + 1) * QSB, kb * KB:(kb + 1) * KB]  # [q, k]
            if not blk.any():
                continue          # fully masked block contributes nothing
            row.append((kb, None if blk.all() else ("dram", qsb, kb)))
        plan.append(row)
    return plan, "generic"


# ------------------------------------------------------------------- builder

def _build(plan, mode):
    nc = bacc.Bacc("TRN2", target_bir_lowering=False, debug=False, num_devices=1)
    AF = mybir.ActivationFunctionType
    OP = mybir.AluOpType

    xT_d = nc.dram_tensor("xT", [NQSB, P, DC, QSB], BF16,
                          kind="ExternalInput").ap()
    wqT_d = nc.dram_tensor("wqT", [4, P, DC, P], BF16,
                           kind="ExternalInput").ap()
    wkT_d = nc.dram_tensor("wkT", [4, P, DC, P], BF16,
                           kind="ExternalInput").ap()
    wvT_d = nc.dram_tensor("wvT", [P, DC, OG], BF16, kind="ExternalInput").ap()
    woT_d = nc.dram_tensor("woT", [8, P, JC, P], BF16,
                           kind="ExternalInput").ap()
    cos_d = nc.dram_tensor("cosT", [P, S], BF16, kind="ExternalInput").ap()
    sin_d = nc.dram_tensor("sinT", [P, S], BF16, kind="ExternalInput").ap()
    psg_d = nc.dram_tensor("psgT", [P, P], BF16, kind="ExternalInput").ap()
    if mode == "causal":
        m01_d = nc.dram_tensor("m01", [P, 4, QSB], BF16, kind="ExternalInput").ap()
    elif mode == "generic":
        m01_d = nc.dram_tensor("m01", [NQSB, NKB, P, QSB], F32,
                               kind="ExternalInput").ap()
    else:
        m01_d = None
    one64_d = nc.dram_tensor("one64", [1, DH], F32R, kind="ExternalInput").ap()
    outT_d = nc.dram_tensor("outT", [D, S], BF16, kind="ExternalOutput").ap()
    outB_d = nc.dram_tensor("outB", [D, S], BF16, kind="ExternalOutput").ap()

    with tile.TileContext(nc) as tc:
        from contextlib import ExitStack
        with ExitStack() as ctx:
            persist = ctx.enter_context(tc.tile_pool(name="persist", bufs=1))
            wstream = ctx.enter_context(tc.tile_pool(name="wstream", bufs=2))
            work = ctx.enter_context(tc.tile_pool(name="work", bufs=2))
            prepool = ctx.enter_context(tc.tile_pool(name="prepool", bufs=2))
            ptpool = ctx.enter_context(tc.tile_pool(name="ptpool", bufs=3))
            stp = ctx.enter_context(
                tc.tile_pool(name="stp", bufs=2, space="PSUM"))
            spp = ctx.enter_context(
                tc.tile_pool(name="spp", bufs=2, space="PSUM"))
            pvp = ctx.enter_context(
                tc.tile_pool(name="pvp", bufs=2, space="PSUM"))

            # bf16 post-rope Q/K and bf16 V (with ones column) live all-kernel
            QTb = [persist.tile([P, S], BF16, tag=f"qt{t}", name=f"qtb{t}")
                   for t in range(4)]
            KTb = [persist.tile([P, S], BF16, tag=f"kt{t}", name=f"ktb{t}")
                   for t in range(4)]
            V = [persist.tile([P, HPG, DH + 1], BF16, tag=f"v{sb}",
                              name=f"v{sb}") for sb in range(NKB)]
            for sb in range(NKB):
                nc.vector.memset(V[sb][:, :, DH:DH + 1], 1.0)

            xTs = [persist.tile([P, DC, QSB], BF16, tag=f"xt{sc}",
                                 name=f"xt{sc}") for sc in range(4)]
            nc.sync.dma_start(xTs[0][:], xT_d[0])
            # prefetch tile-0 Q/K weight chunks ahead of the bulk loads so the
            # first projection matmuls are not stuck behind ~15 MB of DMA
            wqk_live = {}
            for who, w_d in (("q", wqT_d), ("k", wkT_d)):
                w_oc = wstream.tile([P, DC, P], BF16, tag="wqk",
                                    name=f"w{who}0")
                nc.sync.dma_start(w_oc[:], w_d[0])
                wqk_live[who] = w_oc
            psg_sb = persist.tile([P, P], BF16, tag="psg")
            nc.sync.dma_start(psg_sb[:], psg_d)
            wv = persist.tile([P, DC, OG], BF16, tag="wv")
            nc.gpsimd.dma_start(wv[:], wvT_d)
            cos_sb = persist.tile([P, S], BF16, tag="cos")
            sin_sb = persist.tile([P, S], BF16, tag="sin")
            nc.gpsimd.dma_start(cos_sb[:], cos_d)
            nc.gpsimd.dma_start(sin_sb[:], sin_d)
            for sc in range(1, 4):
                nc.sync.dma_start(xTs[sc][:], xT_d[sc])
            aT = [persist.tile([P, S], BF16, tag=f"at{t}", name=f"at{t}")
                  for t in range(4)]
            ones64 = persist.tile([1, DH], F32R, tag="ones64")
            nc.gpsimd.dma_start(ones64[:], one64_d)
            if mode == "causal":
                mk = persist.tile([P, 4, QSB], BF16, tag="m01")
                nc.gpsimd.dma_start(mk[:], m01_d)

            # ---------------- emitters (generators) ----------------
            # yield points let attention pairs and projection halves weave at
            # ~1 us granularity so the PE never sees an ACT-bound stretch

            def gen_qk_unit(w_d, dst, oc, sc, who):
                """One [128, 512] chunk of a Q/K projection + RoPE (2 steps)."""
                if sc == 0 and oc > 0:
                    w_oc = wstream.tile([P, DC, P], BF16, tag="wqk",
                                        name=f"w{who}{oc}")
                    nc.sync.dma_start(w_oc[:], w_d[oc])
                    wqk_live[who] = w_oc
                w_oc = wqk_live[who]
                sl = slice(sc * QSB, (sc + 1) * QSB)
                ps = spp.tile([P, QSB], F32, tag="sp", name="ps")
                for dc in range(4):
                    nc.tensor.matmul(
                        ps[:], w_oc[:, dc, :], xTs[sc][:, dc, :],
                        start=(dc == 0), stop=False)
                yield
                for dc in range(4, DC):
                    nc.tensor.matmul(
                        ps[:], w_oc[:, dc, :], xTs[sc][:, dc, :],
                        start=False, stop=(dc == DC - 1))
                pre = prepool.tile([P, QSB], BF16, tag="pre")
                nc.scalar.copy(pre[:], ps[:])
                rot = spp.tile([P, QSB], F32, tag="sp", name="rot")
                nc.tensor.matmul(rot[:], psg_sb[:], pre[:],
                                 start=True, stop=True)
                m = work.tile([P, QSB], BF16, tag="ropem")
                nc.vector.tensor_tensor(m[:], pre[:], cos_sb[:, sl], OP.mult)
                nc.vector.tensor_tensor(
                    dst[oc][:, sl], rot[:], sin_sb[:, sl], OP.mult)
                nc.vector.tensor_tensor(
                    dst[oc][:, sl], dst[oc][:, sl], m[:], OP.add)
                yield

            def gen_v_unit(sb):
                ps = spp.tile([P, QSB], F32, tag="sp", name="ps")
                xsc, xo = sb // 4, (sb % 4) * P
                for dc in range(4):
                    nc.tensor.matmul(
                        ps[:], xTs[xsc][:, dc, xo:xo + P], wv[:, dc, :],
                        start=(dc == 0), stop=False)
                yield
                for dc in range(4, DC):
                    nc.tensor.matmul(
                        ps[:], xTs[xsc][:, dc, xo:xo + P], wv[:, dc, :],
                        start=False, stop=(dc == DC - 1))
                nc.scalar.copy(
                    V[sb][:, :, 0:DH],
                    ps[:].rearrange("p (h j) -> p h j", j=DH))
                yield

            wos = []

            def gen_op_unit(oc, sc, jlo, jhi, dest):
                """Half of an out-proj psum group (contraction jc in [jlo,jhi))."""
                ssl = slice(sc * QSB, (sc + 1) * QSB)
                ps = spp.tile([P, QSB], F32, tag="sp", name="ps")
                for jc in range(jlo, jhi):
                    nc.tensor.matmul(
                        ps[:], wos[oc][:, jc, :], aT[jc][:, ssl],
                        start=(jc == jlo), stop=(jc == jhi - 1))
                stg = work.tile([P, QSB], BF16, tag="stg", bufs=3, name="stg")
                nc.vector.tensor_copy(stg[:], ps[:])
                nc.sync.dma_start(dest[oc * P:(oc + 1) * P, ssl], stg[:])
                yield

            pending_norm = []

            def flush_norm():
                while pending_norm:
                    pending_norm.pop(0)()

            def gen_attn_group(h, qsb):
                tq = h // 2
                ph = (h % 2) * DH
                qsl = slice(qsb * QSB, (qsb + 1) * QSB)
                q_ap = QTb[tq][ph:ph + DH, qsl]
                blocks = plan[qsb]
                pv = pvp.tile([DH + 1, QSB], F32, tag="pv", name="pv")
                bi = 0
                first = True
                for p0 in range(0, len(blocks), 2):
                    pair = blocks[p0:p0 + 2]
                    w = len(pair)
                    # two k-blocks share one [128, 1024] psum tile (2 banks)
                    # so a single wide Exp covers both
                    st2 = stp.tile([P, 2, QSB], F32, tag="st", name="st2")
                    for j, (kb, msel) in enumerate(pair):
                        nc.tensor.matmul(
                            st2[:, j, :],
                            KTb[tq][ph:ph + DH, kb * KB:(kb + 1) * KB],
                            q_ap, start=True, stop=True)
                    pt2 = ptpool.tile([P, 2, QSB], BF16, tag="pt")
                    diag = [msel for (_, msel) in pair if msel is not None
                            and msel[0] == "const"]
                    if len(diag) == 0:
                        nc.scalar.activation(
                            pt2[:, 0:w, :], st2[:, 0:w, :], AF.Exp,
                            scale=SCALE)
                    else:
                        # exp only the causally-valid columns; fully masked
                        # columns below the diagonal strip are zeroed
                        for j, (kb, msel) in enumerate(pair):
                            lo = 0
                            if msel is not None and msel[0] == "const":
                                lo = KB * msel[1]
                            if lo > 0:
                                nc.vector.memset(pt2[:, j, 0:lo], 0.0)
                            nc.scalar.activation(
                                pt2[:, j, lo:QSB], st2[:, j, lo:QSB],
                                AF.Exp, scale=SCALE)
                    for j, (kb, msel) in enumerate(pair):
                        if msel is not None:
                            if msel[0] == "const":
                                lo = KB * msel[1]
                                nc.vector.tensor_tensor(
                                    pt2[:, j, lo:QSB], pt2[:, j, lo:QSB],
                                    mk[:, msel[1], lo:QSB], OP.mult)
                            else:
                                mg = work.tile([P, QSB], F32, tag="ropem")
                                nc.sync.dma_start(
                                    mg[:], m01_d[msel[1], msel[2]])
                                mgb = ptpool.tile(
                                    [P, 2, QSB], BF16, tag="pt", name="mgb")
                                nc.vector.tensor_copy(mgb[:, 0, :], mg[:])
                                nc.vector.tensor_tensor(
                                    pt2[:, j, :], pt2[:, j, :],
                                    mgb[:, 0, :], OP.mult)
                        nc.tensor.matmul(
                            pv[:], V[kb][:, h, :], pt2[:, j, :],
                            start=(bi == 0), stop=(bi == len(blocks) - 1))
                        bi += 1
                    if first:
                        # run the previous group's deferred normalization now,
                        # one pair into this group, so its PE broadcast hides
                        # behind fresh ST work instead of stalling the stream
                        flush_norm()
                        first = False
                    yield

                def _norm(pv=pv, tq=tq, ph=ph, qsl=qsl):
                    # normalize: broadcast l via PE, 1/ via fast approx,
                    # fused psum*sbuf multiply writes aT directly
                    lrow = work.tile([1, QSB], F32R, tag="nrm", name="lrow",
                                     bufs=3)
                    with nc.allow_low_precision(reason="f32r rounding of l"):
                        nc.vector.tensor_copy(lrow[:], pv[DH:DH + 1, :])
                    bc = spp.tile([P, QSB], F32, tag="sp", name="bc")
                    nc.tensor.matmul(
                        bc[0:DH, :], ones64[:], lrow[:], start=True, stop=True)
                    binv = work.tile([DH, QSB], F32, tag="nrm", name="binv",
                                     bufs=3)
                    nc.vector.reciprocal_approx_fast(binv[:], bc[0:DH, :])
                    nc.vector.tensor_tensor(
                        aT[tq][ph:ph + DH, qsl], pv[0:DH, :], binv[:], OP.mult)
                pending_norm.append(_norm)
                yield

            def drain(g):
                for _ in g:
                    pass

            def chain(gens):
                for g in gens:
                    yield from g

            def weave(agen, pgen, ratio):
                """Drain agen; after each yield, advance pgen by `ratio`."""
                acc = 0.0
                alive = True
                for _ in agen:
                    if not alive:
                        continue
                    acc += ratio
                    while acc >= 1.0:
                        if next(pgen, _SENT) is _SENT:
                            alive = False
                            break
                        acc -= 1.0
                for _ in pgen:
                    pass

            _SENT = object()

            # ---------------- interleaved emission ----------------
            # tile 0 projections + all of V up front (V feeds every round,
            # and trace order defines the dependency semantics); grouped by
            # x chunk so early units only wait on early DMA arrivals
            for sc in range(4):
                drain(gen_qk_unit(wqT_d, QTb, 0, sc, "q"))
                drain(gen_qk_unit(wkT_d, KTb, 0, sc, "k"))
                for sb in range(4 * sc, 4 * sc + 4):
                    drain(gen_v_unit(sb))

            # rounds: attention for head pair t woven with tile t+1
            # projections / (round 3) the first half of the output projection,
            # so the PE always has ACT-free matmul work within a HAM window
            for t in range(4):
                if t == 1:
                    # prefetch all out-proj weights (needed from round 2 on)
                    for oc in range(8):
                        wo = wstream.tile([P, JC, P], BF16, tag="wo", bufs=8,
                                          name=f"wo{oc}")
                        nc.sync.dma_start(wo[:], woT_d[oc])
                        wos.append(wo)
                agen = chain([gen_attn_group(2 * t, qsb)
                              for qsb in range(NQSB)]
                             + [gen_attn_group(2 * t + 1, qsb)
                                for qsb in range(NQSB)])
                pgens = []
                if t < 3:
                    for sc in range(4):
                        pgens.append(gen_qk_unit(wqT_d, QTb, t + 1, sc, "q"))
                    for sc in range(4):
                        pgens.append(gen_qk_unit(wkT_d, KTb, t + 1, sc, "k"))
                    n_steps = 16
                else:
                    for oc in range(8):
                        for sc in range(4):
                            pgens.append(gen_op_unit(oc, sc, 0, 2, outT_d))
                    n_steps = 32
                # ~48 attention yields per round (40 pairs + 8 norms)
                weave(agen, chain(pgens), n_steps / 48.0)

            flush_norm()
            # second half of the output projection (aT[2], aT[3])
            for oc in range(8):
                for sc in range(4):
                    drain(gen_op_unit(oc, sc, 2, JC, outB_d))

    nc.compile()
    return nc


def _plan_key(plan, mode):
    return (mode, tuple(tuple(row) for row in plan))


def _get_compiled(mask):
    plan, mode = _mask_plan(mask)
    key = _plan_key(plan, mode)
    if key not in _COMPILED:
        _COMPILED[key] = (_build(plan, mode), plan, mode)
    return _COMPILED[key]


# --------------------------------------------------------------- host driver

def _make_in_maps(x, Wq, Wk, Wv, Wo, mask, mode):
    cosT2, sinT2, psigT = _host_consts()
    consts = {"cosT": cosT2.astype(ml_dtypes.bfloat16),
              "sinT": sinT2.astype(ml_dtypes.bfloat16),
              "psgT": psigT.astype(ml_dtypes.bfloat16),
              "one64": np.ones((1, DH), np.float32)}
    if mode == "causal":
        m01 = np.zeros((4, P, QSB), np.float32)
        for r in range(4):
            for k in range(P):
                q0 = KB * r + k
                if q0 < QSB:
                    m01[r, k, q0:] = 1.0
        # [P, 4, QSB] pre-arranged for contiguous DMA
        consts["m01"] = np.ascontiguousarray(
            m01.transpose(1, 0, 2)).astype(ml_dtypes.bfloat16)
    elif mode == "generic":
        m = (np.asarray(mask).reshape(S, S) != 0)
        m01 = np.zeros((NQSB, NKB, P, QSB), np.float32)
        for qsb in range(NQSB):
            for kb in range(NKB):
                blk = m[qsb * QSB:(qsb + 1) * QSB, kb * KB:(kb + 1) * KB]
                m01[qsb, kb] = blk.T.astype(np.float32)
        consts["m01"] = m01

    def arr_qk(w):
        # [D, OG_rows] -> per-oc [P, DC, P]: wT[d, o] laid out [oc, p(o), dc, o']
        wT = w.T.astype(np.float32)                       # [D, OG]
        a = wT.reshape(DC, P, 4, P)          # [dc, p(d), oc, o']
        return np.ascontiguousarray(a.transpose(2, 1, 0, 3)).astype(
            ml_dtypes.bfloat16)

    in_maps = []
    for c in range(NCORES):
        b, g = c // HG, c % HG
        rows = slice(OG * g, OG * (g + 1))
        xT = x[b].T.astype(np.float32)                    # [D, S]
        xTa = np.ascontiguousarray(
            xT.reshape(DC, P, NQSB, QSB).transpose(2, 1, 0, 3)).astype(
                ml_dtypes.bfloat16)
        wq = arr_qk(Wq[rows, :])
        wk = arr_qk(Wk[rows, :])
        wvT = np.ascontiguousarray(
            Wv[rows, :].T.astype(np.float32).reshape(DC, P, OG)
            .transpose(1, 0, 2)).astype(ml_dtypes.bfloat16)
        woT = Wo[:, rows].T.astype(np.float32)            # [OG, D]
        woa = np.ascontiguousarray(
            woT.reshape(JC, P, 8, P).transpose(2, 1, 0, 3)
        ).astype(ml_dtypes.bfloat16)
        in_maps.append({
            "xT": xTa,
            "wqT": wq,
            "wkT": wk,
            "wvT": wvT,
            "woT": woa,
            **consts,
        })
    return in_maps


def run(x, Wq, Wk, Wv, Wo, mask, trace=False):
    nc, plan, mode = _get_compiled(mask)
    in_maps = _make_in_maps(x, Wq, Wk, Wv, Wo, mask, mode)
    res = bass_utils.run_bass_kernel_spmd(
        nc, in_maps, core_ids=list(range(NCORES)), trace=trace)
    out = np.empty((B, S, D), np.float32)
    for b in range(B):
        acc = (res.results[2 * b]["outT"].astype(np.float32)
               + res.results[2 * b]["outB"].astype(np.float32)
               + res.results[2 * b + 1]["outT"].astype(np.float32)
               + res.results[2 * b + 1]["outB"].astype(np.float32))
        out[b] = acc.T
    return out, res


def kernel(x, Wq, Wk, Wv, Wo, mask):
    x = np.asarray(x, dtype=np.float32)
    Wq = np.asarray(Wq, dtype=np.float32)
    Wk = np.asarray(Wk, dtype=np.float32)
    Wv = np.asarray(Wv, dtype=np.float32)
    Wo = np.asarray(Wo, dtype=np.float32)
    out, _ = run(x, Wq, Wk, Wv, Wo, mask)
    return out



# revision 13
# speedup vs baseline: 1.0645x; 1.0645x over previous
"""Multi-head attention (RoPE, causal) Trainium2 Bass kernel, 8 NeuronCores.

Problem: x[4,2048,1024] -> MHA(16 heads, head_dim 64, RoPE, causal mask) -> [4,2048,1024]

Sharding (pure data/tensor parallel, no collectives):
  core c -> (batch b = c//2, head-group g = c%2); each head-group = 8 heads = 512 dims.
  Each core computes q/k/v projections for its (batch, head-group), RoPE, attention,
  and a partial output projection (columns of Wo for its head group).
  Host sums the two partial outputs per batch (512-dim contraction split).

Kernel layout tricks:
  - Projections computed in transposed [out_dim, seq] layout (QT/KT) so that
    QK^T blocks come out as S^T [k, q]: softmax reductions along the partition
    dim are avoided entirely via UNSAFE softmax (no row-max; inputs are bounded
    N(0,1)-ish data, logits stay << 88) and the row-sum is folded into the PV
    matmul by augmenting V with a ones column.  No on-chip transposes anywhere.
  - Dual-head attention: the two heads of a 128-row Q/K tile pair occupy
    partitions 0-63 / 64-127.  Their QK^T matmuls contract over only 64
    partitions each, so they are issued back-to-back: the PE row-tiling
    (tile_position auto-derived from base partitions 0 / 64) runs them
    CONCURRENTLY in disjoint quadrant rows -> ~2x effective QK throughput.
  - One Exp activation per k-block covers both heads ([128, 2, 512] PSUM
    pair-tile); causal diagonal blocks only exp/mask/PV the valid q-range
    (no memsets, narrower matmuls).
  - PV accumulates into a [65, 2, 512] PSUM tile (ones column -> row-sums at
    partition 64); at group end the tile is evacuated to SBUF by DVE casts
    (odd head cast crosses quadrants 0->2, HW-supported for 64-row ops) so
    the single PSUM pv buffer recycles immediately; normalization (recip +
    PE broadcast of 1/l + one in-place multiply for both heads) is deferred
    one group to hide behind fresh QK work.
  - Projections / out-proj in bf16; attention QK'/PV in bf16.
  - Output written as [4, 1024, 512] seq-blocks so every store is one fully
    contiguous 128 KB DMA; tail out-proj stores alternate between the sync
    and gpsimd DMA queues to double drain bandwidth.
"""

import numpy as np
import ml_dtypes

import concourse.bass as bass
import concourse.tile as tile
from concourse import bacc, mybir
from concourse import bass_utils

B, S, D, H, DH = 4, 2048, 1024, 16, 64
NCORES = 8
HG = 2              # head groups (tensor parallel)
HPG = H // HG       # heads per group = 8
OG = HPG * DH       # group output dims = 512
SCALE = DH ** -0.5
P = 128
QSB = 512           # q super-block width
NQSB = S // QSB     # 4
KB = 128            # k block
NKB = S // KB       # 16
DC = D // P         # 8 d-chunks
JC = OG // P        # 4 j-chunks (out-proj contraction)

F32 = mybir.dt.float32
F32R = mybir.dt.float32r
BF16 = mybir.dt.bfloat16

_COMPILED = {}


# ---------------------------------------------------------------- host tables

def _rope_tables():
    inv_freq = 1.0 / (10000.0 ** (np.arange(0, DH, 2, dtype=np.float32) / DH))
    t = np.arange(S, dtype=np.float32)
    freqs = np.outer(t, inv_freq).astype(np.float32)      # [S, 32]
    emb = np.concatenate([freqs, freqs], -1)              # [S, 64]
    return np.cos(emb), np.sin(emb)


def _sel2_const():
    # broadcast selector: row 0 -> out partitions 0-63 (l_e), row 32 -> out
    # partitions 64-127 (l_o)
    sel = np.zeros((DH, P), np.float32)
    sel[0, 0:DH] = 1.0
    sel[32, DH:P] = 1.0
    return sel


def _host_consts():
    cos, sin = _rope_tables()                             # [S, 64]
    cosT2 = np.ascontiguousarray(
        np.concatenate([cos.T, cos.T], axis=0), dtype=np.float32)   # [128, S]
    sinT2 = np.ascontiguousarray(
        np.concatenate([sin.T, sin.T], axis=0), dtype=np.float32)
    # signed permutation: rot(x)[i] = -x[i+32] (j<32) else x[i-32], per 64-row head
    psig = np.zeros((P, P), np.float32)
    for i in range(P):
        j = i % DH
        base = (i // DH) * DH
        if j < 32:
            psig[i, base + j + 32] = -1.0
        else:
            psig[i, base + j - 32] = 1.0
    psigT = np.ascontiguousarray(psig.T)
    return cosT2, sinT2, psigT


def _mask_plan(mask):
    """Classify the [S, S] mask into a per-qsb block plan.

    plan[qsb] = list of (kb, msel); msel is None (no mask), ("const", r) for
    the 4 shared causal diagonal tiles, or ("dram", qsb, kb) for generic
    per-block mask tiles.
    """
    m = np.asarray(mask).reshape(S, S) != 0        # [q, k] True = attend
    causal = np.array_equal(m, np.tril(np.ones((S, S), bool)))
    if causal:
        plan = []
        for qsb in range(NQSB):
            row = []
            for kb in range(4 * qsb + 4):
                r = kb - 4 * qsb
                row.append((kb, None if r < 0 else ("const", r)))
            plan.append(row)
        return plan, "causal"
    if m.all():
        return [[(kb, None) for kb in range(NKB)] for _ in range(NQSB)], "full"
    plan = []
    for qsb in range(NQSB):
        row = []
        for kb in range(NKB):
            blk = m[qsb * QSB:(qsb + 1) * QSB, kb * KB:(kb + 1) * KB]  # [q, k]
            if not blk.any():
                continue          # fully masked block contributes nothing
            row.append((kb, None if blk.all() else ("dram", qsb, kb)))
        plan.append(row)
    return plan, "generic"


# ------------------------------------------------------------------- builder

def _build(plan, mode):
    nc = bacc.Bacc("TRN2", target_bir_lowering=False, debug=False, num_devices=1)
    AF = mybir.ActivationFunctionType
    OP = mybir.AluOpType

    xT_d = nc.dram_tensor("xT", [NQSB, P, DC, QSB], BF16,
                          kind="ExternalInput").ap()
    wqT_d = nc.dram_tensor("wqT", [4, P, DC, P], BF16,
                           kind="ExternalInput").ap()
    wkT_d = nc.dram_tensor("wkT", [4, P, DC, P], BF16,
                           kind="ExternalInput").ap()
    wvT_d = nc.dram_tensor("wvT", [P, DC, OG], BF16, kind="ExternalInput").ap()
    woT_d = nc.dram_tensor("woT", [8, P, JC, P], BF16,
                           kind="ExternalInput").ap()
    cos_d = nc.dram_tensor("cosT", [P, S], BF16, kind="ExternalInput").ap()
    sin_d = nc.dram_tensor("sinT", [P, S], BF16, kind="ExternalInput").ap()
    psg_d = nc.dram_tensor("psgT", [P, P], BF16, kind="ExternalInput").ap()
    if mode == "causal":
        m01_d = nc.dram_tensor("m01", [P, 4, QSB], BF16, kind="ExternalInput").ap()
    elif mode == "generic":
        m01_d = nc.dram_tensor("m01", [NQSB, NKB, P, QSB], F32,
                               kind="ExternalInput").ap()
    else:
        m01_d = None
    one64_d = nc.dram_tensor("one64", [1, DH], F32R, kind="ExternalInput").ap()
    sel2_d = nc.dram_tensor("sel2", [DH, P], F32R, kind="ExternalInput").ap()
    # [sc, d, q] blocked outputs: every [128, 512] store is one contiguous
    # 128 KB DMA (host re-assembles to [D, S])
    outT_d = nc.dram_tensor("outT", [NQSB, D, QSB], BF16,
                            kind="ExternalOutput").ap()
    outB_d = nc.dram_tensor("outB", [NQSB, D, QSB], BF16,
                            kind="ExternalOutput").ap()

    with tile.TileContext(nc) as tc:
        from contextlib import ExitStack
        with ExitStack() as ctx:
            persist = ctx.enter_context(tc.tile_pool(name="persist", bufs=1))
            wstream = ctx.enter_context(tc.tile_pool(name="wstream", bufs=2))
            work = ctx.enter_context(tc.tile_pool(name="work", bufs=2))
            prepool = ctx.enter_context(tc.tile_pool(name="prepool", bufs=2))
            ptpool = ctx.enter_context(tc.tile_pool(name="ptpool", bufs=3))
            # PSUM budget (8 banks): stp 2x[128,2,512] = 4, spp 2x[128,512]
            # = 2, pvp 1x[65,2,512] = 2
            stp = ctx.enter_context(
                tc.tile_pool(name="stp", bufs=2, space="PSUM"))
            spp = ctx.enter_context(
                tc.tile_pool(name="spp", bufs=2, space="PSUM"))
            pvp = ctx.enter_context(
                tc.tile_pool(name="pvp", bufs=1, space="PSUM"))

            # bf16 post-rope Q/K and bf16 V (with ones column) live all-kernel
            QTb = [persist.tile([P, S], BF16, tag=f"qt{t}", name=f"qtb{t}")
                   for t in range(4)]
            KTb = [persist.tile([P, S], BF16, tag=f"kt{t}", name=f"ktb{t}")
                   for t in range(4)]
            V = [persist.tile([P, HPG, DH + 1], BF16, tag=f"v{sb}",
                              name=f"v{sb}") for sb in range(NKB)]
            for sb in range(NKB):
                nc.vector.memset(V[sb][:, :, DH:DH + 1], 1.0)

            xTs = [persist.tile([P, DC, QSB], BF16, tag=f"xt{sc}",
                                 name=f"xt{sc}") for sc in range(4)]
            # prefetch tile-0 Q/K weight chunks + first x half ahead of the
            # bulk loads so the first projection matmuls start ASAP
            wqk_live = {}
            for who, w_d in (("q", wqT_d), ("k", wkT_d)):
                w_oc = wstream.tile([P, DC, P], BF16, tag="wqk",
                                    name=f"w{who}0")
                nc.sync.dma_start(w_oc[:], w_d[0])
                wqk_live[who] = w_oc
            nc.sync.dma_start(xTs[0][:, 0:4, :], xT_d[0][:, 0:4, :])
            nc.sync.dma_start(xTs[0][:, 4:DC, :], xT_d[0][:, 4:DC, :])
            psg_sb = persist.tile([P, P], BF16, tag="psg")
            nc.sync.dma_start(psg_sb[:], psg_d)
            wv = persist.tile([P, DC, OG], BF16, tag="wv")
            nc.gpsimd.dma_start(wv[:], wvT_d)
            cos_sb = persist.tile([P, S], BF16, tag="cos")
            sin_sb = persist.tile([P, S], BF16, tag="sin")
            nc.gpsimd.dma_start(cos_sb[:], cos_d)
            nc.gpsimd.dma_start(sin_sb[:], sin_d)
            for sc in range(1, 4):
                nc.sync.dma_start(xTs[sc][:], xT_d[sc])
            aT = [persist.tile([P, S], BF16, tag=f"at{t}", name=f"at{t}")
                  for t in range(4)]
            ones64 = persist.tile([1, DH], F32R, tag="ones64")
            nc.gpsimd.dma_start(ones64[:], one64_d)
            sel2 = persist.tile([DH, P], F32R, tag="sel2")
            nc.gpsimd.dma_start(sel2[:], sel2_d)
            # l staging rows: l_e at partition 0, l_o at partition 32 (DVE
            # partition writes must be 32-aligned); memset once so the unused
            # rows the broadcast matmul reads are never NaN
            lrows = [persist.tile([DH, QSB], F32R, tag=f"lr{i}",
                                  name=f"lrows{i}") for i in range(2)]
            for i in range(2):
                nc.vector.memset(lrows[i][:].bitcast(F32), 1.0)
            if mode == "causal":
                mk = persist.tile([P, 4, QSB], BF16, tag="m01")
                nc.gpsimd.dma_start(mk[:], m01_d)

            # ---------------- emitters (generators) ----------------
            # yield points let attention steps and projection halves weave at
            # ~1 us granularity so the PE never sees an ACT-bound stretch

            def gen_qk_unit(w_d, dst, oc, sc, who):
                """One [128, 512] chunk of a Q/K projection + RoPE (2 steps)."""
                if sc == 0 and oc > 0:
                    w_oc = wstream.tile([P, DC, P], BF16, tag="wqk",
                                        name=f"w{who}{oc}")
                    nc.sync.dma_start(w_oc[:], w_d[oc])
                    wqk_live[who] = w_oc
                w_oc = wqk_live[who]
                sl = slice(sc * QSB, (sc + 1) * QSB)
                ps = spp.tile([P, QSB], F32, tag="sp", name="ps")
                for dc in range(4):
                    nc.tensor.matmul(
                        ps[:], w_oc[:, dc, :], xTs[sc][:, dc, :],
                        start=(dc == 0), stop=False)
                yield
                for dc in range(4, DC):
                    nc.tensor.matmul(
                        ps[:], w_oc[:, dc, :], xTs[sc][:, dc, :],
                        start=False, stop=(dc == DC - 1))
                pre = prepool.tile([P, QSB], BF16, tag="pre")
                nc.scalar.copy(pre[:], ps[:])
                rot = spp.tile([P, QSB], F32, tag="sp", name="rot")
                nc.tensor.matmul(rot[:], psg_sb[:], pre[:],
                                 start=True, stop=True)
                m = work.tile([P, QSB], BF16, tag="ropem")
                nc.vector.tensor_tensor(m[:], pre[:], cos_sb[:, sl], OP.mult)
                nc.vector.tensor_tensor(
                    dst[oc][:, sl], rot[:], sin_sb[:, sl], OP.mult)
                nc.vector.tensor_tensor(
                    dst[oc][:, sl], dst[oc][:, sl], m[:], OP.add)
                yield

            def gen_v_unit(sb):
                ps = spp.tile([P, QSB], F32, tag="sp", name="ps")
                xsc, xo = sb // 4, (sb % 4) * P
                for dc in range(4):
                    nc.tensor.matmul(
                        ps[:], xTs[xsc][:, dc, xo:xo + P], wv[:, dc, :],
                        start=(dc == 0), stop=False)
                yield
                for dc in range(4, DC):
                    nc.tensor.matmul(
                        ps[:], xTs[xsc][:, dc, xo:xo + P], wv[:, dc, :],
                        start=False, stop=(dc == DC - 1))
                nc.scalar.copy(
                    V[sb][:, :, 0:DH],
                    ps[:].rearrange("p (h j) -> p h j", j=DH))
                yield

            wos = []

            def gen_op_unit(oc, sc, jlo, jhi, dest, dma_eng):
                """Half of an out-proj psum group (contraction jc in [jlo,jhi))."""
                ps = spp.tile([P, QSB], F32, tag="sp", name="ps")
                for jc in range(jlo, jhi):
                    nc.tensor.matmul(
                        ps[:], wos[oc][:, jc, :],
                        aT[jc][:, sc * QSB:(sc + 1) * QSB],
                        start=(jc == jlo), stop=(jc == jhi - 1))
                stg = work.tile([P, QSB], BF16, tag="stg", bufs=3, name="stg")
                nc.vector.tensor_copy(stg[:], ps[:])
                dma_eng.dma_start(dest[sc, oc * P:(oc + 1) * P, :], stg[:])
                yield

            pending_norm = []

            def flush_norm():
                while pending_norm:
                    pending_norm.pop(0)()

            def gen_attn_dual(tq, qsb):
                """Attention for head pair (2tq, 2tq+1) over one q super-block.

                Per k-block: dual row-tiled QK (concurrent), one Exp over both
                heads, causal mask on the valid range only, sequential PV.
                """
                he, ho = 2 * tq, 2 * tq + 1
                qsl = slice(qsb * QSB, (qsb + 1) * QSB)
                blocks = plan[qsb]
                nblk = len(blocks)
                pv = pvp.tile([DH + 1, 2, QSB], F32, tag="pv", name="pv")
                for bi, (kb, msel) in enumerate(blocks):
                    lo = 0
                    if msel is not None and msel[0] == "const":
                        lo = KB * msel[1]
                    ksl = slice(kb * KB, (kb + 1) * KB)
                    qlo = slice(qsb * QSB + lo, (qsb + 1) * QSB)
                    st2 = stp.tile([P, 2, QSB], F32, tag="st", name="st2")
                    # dual row-tiled QK: base partitions 0 / 64 -> disjoint
                    # PE quadrant rows -> concurrent execution
                    nc.tensor.matmul(
                        st2[:, 0, lo:QSB], KTb[tq][0:DH, ksl],
                        QTb[tq][0:DH, qlo], start=True, stop=True)
                    nc.tensor.matmul(
                        st2[:, 1, lo:QSB], KTb[tq][DH:P, ksl],
                        QTb[tq][DH:P, qlo], start=True, stop=True)
                    pt2 = ptpool.tile([P, 2, QSB], BF16, tag="pt")
                    nc.scalar.activation(
                        pt2[:, :, lo:QSB], st2[:, :, lo:QSB], AF.Exp,
                        scale=SCALE)
                    if msel is not None:
                        if msel[0] == "const":
                            r = msel[1]
                            for j in range(2):
                                nc.vector.tensor_tensor(
                                    pt2[:, j, lo:QSB], pt2[:, j, lo:QSB],
                                    mk[:, r, lo:QSB], OP.mult)
                        else:
                            mg = work.tile([P, QSB], F32, tag="ropem")
                            nc.sync.dma_start(mg[:], m01_d[msel[1], msel[2]])
                            mgb = ptpool.tile(
                                [P, 2, QSB], BF16, tag="pt", name="mgb")
                            nc.vector.tensor_copy(mgb[:, 0, :], mg[:])
                            for j in range(2):
                                nc.vector.tensor_tensor(
                                    pt2[:, j, :], pt2[:, j, :],
                                    mgb[:, 0, :], OP.mult)
                    nc.tensor.matmul(
                        pv[:, 0, lo:QSB], V[kb][:, he, :], pt2[:, 0, lo:QSB],
                        start=(bi == 0), stop=(bi == nblk - 1))
                    nc.tensor.matmul(
                        pv[:, 1, lo:QSB], V[kb][:, ho, :], pt2[:, 1, lo:QSB],
                        start=(bi == 0), stop=(bi == nblk - 1))
                    if bi == 0:
                        # run the previous group's deferred normalization now,
                        # one block into this group, so its PE broadcast hides
                        # behind fresh QK work instead of stalling the stream
                        flush_norm()
                    yield

                # evacuate pv to SBUF so the single PSUM pv buffer recycles:
                # row sums (partition 64, banks 0/1) -> partitions 0/32 of
                # the rotating lrow tile; head dims -> aT (odd head crosses
                # quadrants 0->2; 64-row DVE ops support cross-quadrant writes)
                lrow2 = lrows[(tq * NQSB + qsb) % 2]
                with nc.allow_low_precision(reason="f32r rounding of l"):
                    nc.vector.tensor_copy(lrow2[0:1, :], pv[DH:DH + 1, 0, :])
                    nc.vector.tensor_copy(lrow2[32:33, :], pv[DH:DH + 1, 1, :])
                with nc.allow_low_precision(reason="bf16 aT evac"):
                    nc.vector.tensor_copy(aT[tq][0:DH, qsl], pv[0:DH, 0, :])
                    nc.vector.tensor_copy(aT[tq][DH:P, qsl], pv[0:DH, 1, :])

                def _norm(tq=tq, qsl=qsl, lrow2=lrow2):
                    # one PE matmul broadcasts l_e to partitions 0-63 and
                    # l_o to 64-127 (sel2 selector), 1/ via fast approx on
                    # the full 128-row tile, one in-place mult for both heads
                    bc = spp.tile([P, QSB], F32, tag="sp", name="bc")
                    nc.tensor.matmul(bc[:], sel2[:], lrow2[:],
                                     start=True, stop=True)
                    binv = work.tile([P, QSB], F32, tag="nrm2",
                                     name="binv", bufs=2)
                    nc.vector.reciprocal_approx_fast(binv[:], bc[:])
                    nc.vector.tensor_tensor(
                        aT[tq][:, qsl], aT[tq][:, qsl], binv[:], OP.mult)
                pending_norm.append(_norm)
                yield

            def drain(g):
                for _ in g:
                    pass

            def chain(gens):
                for g in gens:
                    yield from g

            def weave(agen, pgen, ratio):
                """Drain agen; after each yield, advance pgen by `ratio`."""
                acc = 0.0
                alive = True
                for _ in agen:
                    if not alive:
                        continue
                    acc += ratio
                    while acc >= 1.0:
                        if next(pgen, _SENT) is _SENT:
                            alive = False
                            break
                        acc -= 1.0
                for _ in pgen:
                    pass

            _SENT = object()

            # ---------------- interleaved emission ----------------
            # tile 0 projections + all of V up front (V feeds every round,
            # and trace order defines the dependency semantics); grouped by
            # x chunk so early units only wait on early DMA arrivals
            for sc in range(4):
                drain(gen_qk_unit(wqT_d, QTb, 0, sc, "q"))
                drain(gen_qk_unit(wkT_d, KTb, 0, sc, "k"))
                for sb in range(4 * sc, 4 * sc + 4):
                    drain(gen_v_unit(sb))

            # rounds: dual-head attention for tile t woven with tile t+1
            # projections / (round 3) the first half of the output projection,
            # so the PE always has ACT-free matmul work within a HAM window
            n_attn_yields = sum(len(row) for row in plan) + NQSB
            for t in range(4):
                if t == 1:
                    # prefetch all out-proj weights (needed from round 3 on)
                    for oc in range(8):
                        wo = wstream.tile([P, JC, P], BF16, tag="wo", bufs=8,
                                          name=f"wo{oc}")
                        nc.sync.dma_start(wo[:], woT_d[oc])
                        wos.append(wo)
                agen = chain([gen_attn_dual(t, qsb) for qsb in range(NQSB)])
                pgens = []
                if t < 3:
                    for sc in range(4):
                        pgens.append(gen_qk_unit(wqT_d, QTb, t + 1, sc, "q"))
                    for sc in range(4):
                        pgens.append(gen_qk_unit(wkT_d, KTb, t + 1, sc, "k"))
                    n_steps = 16
                else:
                    for oc in range(8):
                        for sc in range(4):
                            pgens.append(
                                gen_op_unit(oc, sc, 0, 2, outT_d, nc.sync))
                    n_steps = 32
                weave(agen, chain(pgens), n_steps / n_attn_yields)

            flush_norm()
            # second half of the output projection (aT[2], aT[3]); stores
            # alternate between the sync and gpsimd DMA queues
            for oc in range(8):
                for sc in range(4):
                    eng = nc.gpsimd if (oc + sc) % 2 == 0 else nc.sync
                    drain(gen_op_unit(oc, sc, 2, JC, outB_d, eng))

    nc.compile()
    return nc


def _plan_key(plan, mode):
    return (mode, tuple(tuple(row) for row in plan))


def _get_compiled(mask):
    plan, mode = _mask_plan(mask)
    key = _plan_key(plan, mode)
    if key not in _COMPILED:
        _COMPILED[key] = (_build(plan, mode), plan, mode)
    return _COMPILED[key]


# --------------------------------------------------------------- host driver

def _make_in_maps(x, Wq, Wk, Wv, Wo, mask, mode):
    cosT2, sinT2, psigT = _host_consts()
    consts = {"cosT": cosT2.astype(ml_dtypes.bfloat16),
              "sinT": sinT2.astype(ml_dtypes.bfloat16),
              "psgT": psigT.astype(ml_dtypes.bfloat16),
              "one64": np.ones((1, DH), np.float32),
              "sel2": _sel2_const()}
    if mode == "causal":
        m01 = np.zeros((4, P, QSB), np.float32)
        for r in range(4):
            for k in range(P):
                q0 = KB * r + k
                if q0 < QSB:
                    m01[r, k, q0:] = 1.0
        # [P, 4, QSB] pre-arranged for contiguous DMA
        consts["m01"] = np.ascontiguousarray(
            m01.transpose(1, 0, 2)).astype(ml_dtypes.bfloat16)
    elif mode == "generic":
        m = (np.asarray(mask).reshape(S, S) != 0)
        m01 = np.zeros((NQSB, NKB, P, QSB), np.float32)
        for qsb in range(NQSB):
            for kb in range(NKB):
                blk = m[qsb * QSB:(qsb + 1) * QSB, kb * KB:(kb + 1) * KB]
                m01[qsb, kb] = blk.T.astype(np.float32)
        consts["m01"] = m01

    def arr_qk(w):
        # [D, OG_rows] -> per-oc [P, DC, P]: wT[d, o] laid out [oc, p(o), dc, o']
        wT = w.T.astype(np.float32)                       # [D, OG]
        a = wT.reshape(DC, P, 4, P)          # [dc, p(d), oc, o']
        return np.ascontiguousarray(a.transpose(2, 1, 0, 3)).astype(
            ml_dtypes.bfloat16)

    in_maps = []
    for c in range(NCORES):
        b, g = c // HG, c % HG
        rows = slice(OG * g, OG * (g + 1))
        xT = x[b].T.astype(np.float32)                    # [D, S]
        xTa = np.ascontiguousarray(
            xT.reshape(DC, P, NQSB, QSB).transpose(2, 1, 0, 3)).astype(
                ml_dtypes.bfloat16)
        wq = arr_qk(Wq[rows, :])
        wk = arr_qk(Wk[rows, :])
        wvT = np.ascontiguousarray(
            Wv[rows, :].T.astype(np.float32).reshape(DC, P, OG)
            .transpose(1, 0, 2)).astype(ml_dtypes.bfloat16)
        woT = Wo[:, rows].T.astype(np.float32)            # [OG, D]
        woa = np.ascontiguousarray(
            woT.reshape(JC, P, 8, P).transpose(2, 1, 0, 3)
        ).astype(ml_dtypes.bfloat16)
        in_maps.append({
            "xT": xTa,
            "wqT": wq,
            "wkT": wk,
            "wvT": wvT,
            "woT": woa,
            **consts,
        })
    return in_maps


def run(x, Wq, Wk, Wv, Wo, mask, trace=False):
    nc, plan, mode = _get_compiled(mask)
    in_maps = _make_in_maps(x, Wq, Wk, Wv, Wo, mask, mode)
    res = bass_utils.run_bass_kernel_spmd(
        nc, in_maps, core_ids=list(range(NCORES)), trace=trace)
    out = np.empty((B, S, D), np.float32)
    for b in range(B):
        acc = (res.results[2 * b]["outT"].astype(np.float32)
               + res.results[2 * b]["outB"].astype(np.float32)
               + res.results[2 * b + 1]["outT"].astype(np.float32)
               + res.results[2 * b + 1]["outB"].astype(np.float32))
        # [NQSB, D, QSB] -> [D, S] -> [S, D]
        out[b] = acc.transpose(1, 0, 2).reshape(D, S).T
    return out, res


def kernel(x, Wq, Wk, Wv, Wo, mask):
    x = np.asarray(x, dtype=np.float32)
    Wq = np.asarray(Wq, dtype=np.float32)
    Wk = np.asarray(Wk, dtype=np.float32)
    Wv = np.asarray(Wv, dtype=np.float32)
    Wo = np.asarray(Wo, dtype=np.float32)
    out, _ = run(x, Wq, Wk, Wv, Wo, mask)
    return out


# revision 20
# speedup vs baseline: 1.0776x; 1.0122x over previous
"""Multi-head attention (RoPE, causal) Trainium2 Bass kernel, 8 NeuronCores.

Problem: x[4,2048,1024] -> MHA(16 heads, head_dim 64, RoPE, causal mask) -> [4,2048,1024]

Sharding (pure data/tensor parallel, no collectives):
  core c -> (batch b = c//2, head-group g = c%2); each head-group = 8 heads = 512 dims.
  Each core computes q/k/v projections for its (batch, head-group), RoPE, attention,
  and a partial output projection (columns of Wo for its head group).
  Host sums the two partial outputs per batch (512-dim contraction split).

Kernel layout tricks:
  - Projections computed in transposed [out_dim, seq] layout (QT/KT) so that
    QK^T blocks come out as S^T [k, q]: softmax reductions along the partition
    dim are avoided entirely via UNSAFE softmax (no row-max; inputs are bounded
    N(0,1)-ish data, logits stay << 88) and the row-sum is folded into the PV
    matmul by augmenting V with a ones column.  No on-chip transposes anywhere.
  - Dual-head attention: the two heads of a 128-row Q/K tile pair occupy
    partitions 0-63 / 64-127.  Their QK^T matmuls contract over only 64
    partitions each, so they are issued back-to-back: the PE row-tiling
    (tile_position auto-derived from base partitions 0 / 64) runs them
    CONCURRENTLY in disjoint quadrant rows -> ~2x effective QK throughput.
  - One Exp activation per k-block covers both heads ([128, 2, 512] PSUM
    pair-tile); causal diagonal blocks only exp/mask/PV the valid q-range
    (no memsets, narrower matmuls).
  - PV accumulates into a [65, 2, 512] PSUM tile (ones column -> row-sums at
    partition 64); at group end the tile is evacuated to SBUF by DVE casts
    (odd head cast crosses quadrants 0->2, HW-supported for 64-row ops) so
    the single PSUM pv buffer recycles immediately; normalization (recip +
    PE broadcast of 1/l + one in-place multiply for both heads) is deferred
    one group to hide behind fresh QK work.
  - Projections / out-proj in bf16; attention QK'/PV in bf16.
  - Output written as [4, 1024, 512] seq-blocks so every store is one fully
    contiguous 128 KB DMA; tail out-proj stores alternate between the sync
    and gpsimd DMA queues to double drain bandwidth.
"""

import numpy as np
import ml_dtypes

import concourse.bass as bass
import concourse.tile as tile
from concourse import bacc, mybir
from concourse import bass_utils

B, S, D, H, DH = 4, 2048, 1024, 16, 64
NCORES = 8
HG = 2              # head groups (tensor parallel)
HPG = H // HG       # heads per group = 8
OG = HPG * DH       # group output dims = 512
SCALE = DH ** -0.5
P = 128
QSB = 512           # q super-block width
NQSB = S // QSB     # 4
KB = 128            # k block
NKB = S // KB       # 16
DC = D // P         # 8 d-chunks
JC = OG // P        # 4 j-chunks (out-proj contraction)

F32 = mybir.dt.float32
F32R = mybir.dt.float32r
BF16 = mybir.dt.bfloat16

_COMPILED = {}


# ---------------------------------------------------------------- host tables

def _rope_tables():
    inv_freq = 1.0 / (10000.0 ** (np.arange(0, DH, 2, dtype=np.float32) / DH))
    t = np.arange(S, dtype=np.float32)
    freqs = np.outer(t, inv_freq).astype(np.float32)      # [S, 32]
    emb = np.concatenate([freqs, freqs], -1)              # [S, 64]
    return np.cos(emb), np.sin(emb)


def _sel2_const():
    # broadcast selector: row 0 -> out partitions 0-63 (l_e), row 32 -> out
    # partitions 64-127 (l_o)
    sel = np.zeros((DH, P), np.float32)
    sel[0, 0:DH] = 1.0
    sel[32, DH:P] = 1.0
    return sel


def _host_consts():
    cos, sin = _rope_tables()                             # [S, 64]
    cosT2 = np.ascontiguousarray(
        np.concatenate([cos.T, cos.T], axis=0), dtype=np.float32)   # [128, S]
    sinT2 = np.ascontiguousarray(
        np.concatenate([sin.T, sin.T], axis=0), dtype=np.float32)
    # signed permutation: rot(x)[i] = -x[i+32] (j<32) else x[i-32], per 64-row head
    psig = np.zeros((P, P), np.float32)
    for i in range(P):
        j = i % DH
        base = (i // DH) * DH
        if j < 32:
            psig[i, base + j + 32] = -1.0
        else:
            psig[i, base + j - 32] = 1.0
    psigT = np.ascontiguousarray(psig.T)
    return cosT2, sinT2, psigT


def _mask_plan(mask):
    """Classify the [S, S] mask into a per-qsb block plan.

    plan[qsb] = list of (kb, msel); msel is None (no mask), ("const", r) for
    the 4 shared causal diagonal tiles, or ("dram", qsb, kb) for generic
    per-block mask tiles.
    """
    m = np.asarray(mask).reshape(S, S) != 0        # [q, k] True = attend
    causal = np.array_equal(m, np.tril(np.ones((S, S), bool)))
    if causal:
        plan = []
        for qsb in range(NQSB):
            row = []
            for kb in range(4 * qsb + 4):
                r = kb - 4 * qsb
                row.append((kb, None if r < 0 else ("const", r)))
            plan.append(row)
        return plan, "causal"
    if m.all():
        return [[(kb, None) for kb in range(NKB)] for _ in range(NQSB)], "full"
    plan = []
    for qsb in range(NQSB):
        row = []
        for kb in range(NKB):
            blk = m[qsb * QSB:(qsb + 1) * QSB, kb * KB:(kb + 1) * KB]  # [q, k]
            if not blk.any():
                continue          # fully masked block contributes nothing
            row.append((kb, None if blk.all() else ("dram", qsb, kb)))
        plan.append(row)
    return plan, "generic"


# ------------------------------------------------------------------- builder

def _build(plan, mode):
    nc = bacc.Bacc("TRN2", target_bir_lowering=False, debug=False, num_devices=1)
    AF = mybir.ActivationFunctionType
    OP = mybir.AluOpType

    xT_d = nc.dram_tensor("xT", [NQSB, P, DC, QSB], BF16,
                          kind="ExternalInput").ap()
    wqT_d = nc.dram_tensor("wqT", [4, P, DC, P], BF16,
                           kind="ExternalInput").ap()
    wkT_d = nc.dram_tensor("wkT", [4, P, DC, P], BF16,
                           kind="ExternalInput").ap()
    wvT_d = nc.dram_tensor("wvT", [P, DC, OG], BF16, kind="ExternalInput").ap()
    woT_d = nc.dram_tensor("woT", [8, P, JC, P], BF16,
                           kind="ExternalInput").ap()
    cos_d = nc.dram_tensor("cosT", [P, S], BF16, kind="ExternalInput").ap()
    sin_d = nc.dram_tensor("sinT", [P, S], BF16, kind="ExternalInput").ap()
    psg_d = nc.dram_tensor("psgT", [P, P], BF16, kind="ExternalInput").ap()
    if mode == "causal":
        m01_d = nc.dram_tensor("m01", [P, 4, QSB], BF16, kind="ExternalInput").ap()
    elif mode == "generic":
        m01_d = nc.dram_tensor("m01", [NQSB, NKB, P, QSB], F32,
                               kind="ExternalInput").ap()
    else:
        m01_d = None
    one64_d = nc.dram_tensor("one64", [1, DH], F32R, kind="ExternalInput").ap()
    sel2_d = nc.dram_tensor("sel2", [DH, P], F32R, kind="ExternalInput").ap()
    # [sc, p, oc, q] blocked outputs: one 1 MB DMA per seq-block with fully
    # contiguous 8 KB per-partition lines (host re-assembles to [D, S])
    outT_d = nc.dram_tensor("outT", [NQSB, P, 8, QSB], BF16,
                            kind="ExternalOutput").ap()
    outB_d = nc.dram_tensor("outB", [NQSB, P, 8, QSB], BF16,
                            kind="ExternalOutput").ap()

    with tile.TileContext(nc) as tc:
        from contextlib import ExitStack
        with ExitStack() as ctx:
            persist = ctx.enter_context(tc.tile_pool(name="persist", bufs=1))
            wstream = ctx.enter_context(tc.tile_pool(name="wstream", bufs=2))
            work = ctx.enter_context(tc.tile_pool(name="work", bufs=2))
            prepool = ctx.enter_context(tc.tile_pool(name="prepool", bufs=2))
            ptpool = ctx.enter_context(tc.tile_pool(name="ptpool", bufs=3))
            outp = ctx.enter_context(tc.tile_pool(name="outp", bufs=2))
            # PSUM budget (8 banks): stp 2x[128,2,512] = 4, spp 2x[128,512]
            # = 2, pvp 1x[65,2,512] = 2
            stp = ctx.enter_context(
                tc.tile_pool(name="stp", bufs=2, space="PSUM"))
            spp = ctx.enter_context(
                tc.tile_pool(name="spp", bufs=2, space="PSUM"))
            pvp = ctx.enter_context(
                tc.tile_pool(name="pvp", bufs=1, space="PSUM"))

            # bf16 post-rope Q/K and bf16 V (with ones column) live all-kernel
            QTb = [persist.tile([P, S], BF16, tag=f"qt{t}", name=f"qtb{t}")
                   for t in range(4)]
            KTb = [persist.tile([P, S], BF16, tag=f"kt{t}", name=f"ktb{t}")
                   for t in range(4)]
            V = [persist.tile([P, HPG, DH + 1], BF16, tag=f"v{sb}",
                              name=f"v{sb}") for sb in range(NKB)]
            for sb in range(NKB):
                nc.vector.memset(V[sb][:, :, DH:DH + 1], 1.0)

            xTs = [persist.tile([P, DC, QSB], BF16, tag=f"xt{sc}",
                                 name=f"xt{sc}") for sc in range(4)]
            # prefetch tile-0 Q/K weight chunks + first x half ahead of the
            # bulk loads so the first projection matmuls start ASAP
            wqk_live = {}
            for who, w_d in (("q", wqT_d), ("k", wkT_d)):
                w_oc = wstream.tile([P, DC, P], BF16, tag="wqk",
                                    name=f"w{who}0")
                nc.sync.dma_start(w_oc[:], w_d[0])
                wqk_live[who] = w_oc
            nc.sync.dma_start(xTs[0][:, 0:4, :], xT_d[0][:, 0:4, :])
            nc.sync.dma_start(xTs[0][:, 4:DC, :], xT_d[0][:, 4:DC, :])
            psg_sb = persist.tile([P, P], BF16, tag="psg")
            nc.sync.dma_start(psg_sb[:], psg_d)
            wv = persist.tile([P, DC, OG], BF16, tag="wv")
            nc.gpsimd.dma_start(wv[:], wvT_d)
            cos_sb = persist.tile([P, S], BF16, tag="cos")
            sin_sb = persist.tile([P, S], BF16, tag="sin")
            nc.gpsimd.dma_start(cos_sb[:], cos_d)
            nc.gpsimd.dma_start(sin_sb[:], sin_d)
            for sc in range(1, 4):
                nc.sync.dma_start(xTs[sc][:], xT_d[sc])
            aT = [persist.tile([P, S], BF16, tag=f"at{t}", name=f"at{t}")
                  for t in range(4)]
            ones64 = persist.tile([1, DH], F32R, tag="ones64")
            nc.gpsimd.dma_start(ones64[:], one64_d)
            sel2 = persist.tile([DH, P], F32R, tag="sel2")
            nc.gpsimd.dma_start(sel2[:], sel2_d)
            # l staging rows: l_e at partition 0, l_o at partition 32 (DVE
            # partition writes must be 32-aligned); memset once so the unused
            # rows the broadcast matmul reads are never NaN
            lrows = [persist.tile([DH, QSB], F32R, tag=f"lr{i}",
                                  name=f"lrows{i}") for i in range(2)]
            for i in range(2):
                nc.vector.memset(lrows[i][:].bitcast(F32), 1.0)
            if mode == "causal":
                mk = persist.tile([P, 4, QSB], BF16, tag="m01")
                nc.gpsimd.dma_start(mk[:], m01_d)

            # ---------------- emitters (generators) ----------------
            # yield points let attention steps and projection halves weave at
            # ~1 us granularity so the PE never sees an ACT-bound stretch

            def gen_qk_unit(w_d, dst, oc, sc, who, woven=False):
                """One [128, 512] chunk of a Q/K projection + RoPE.

                Fine-grained yields (2 matmuls per step) keep PE insertions
                between attention steps small so the Exp stream never starves.
                """
                if sc == 0 and oc > 0:
                    w_oc = wstream.tile([P, DC, P], BF16, tag="wqk",
                                        name=f"w{who}{oc}")
                    nc.sync.dma_start(w_oc[:], w_d[oc])
                    wqk_live[who] = w_oc
                w_oc = wqk_live[who]
                sl = slice(sc * QSB, (sc + 1) * QSB)
                ps = spp.tile([P, QSB], F32, tag="sp", name="ps")
                for dc in range(DC):
                    nc.tensor.matmul(
                        ps[:], w_oc[:, dc, :], xTs[sc][:, dc, :],
                        start=(dc == 0), stop=(dc == DC - 1))
                    if dc % 2 == 1 and dc < DC - 1:
                        yield
                pre = prepool.tile([P, QSB], BF16, tag="pre")
                # pre copy off ScalarE during attention rounds (Exp is the
                # critical engine there); ScalarE when woven upfront
                if woven:
                    nc.vector.tensor_copy(pre[:], ps[:])
                else:
                    nc.scalar.copy(pre[:], ps[:])
                yield
                rot = spp.tile([P, QSB], F32, tag="sp", name="rot")
                nc.tensor.matmul(rot[:], psg_sb[:], pre[:],
                                 start=True, stop=True)
                m = work.tile([P, QSB], BF16, tag="ropem")
                nc.vector.tensor_tensor(m[:], pre[:], cos_sb[:, sl], OP.mult)
                nc.vector.tensor_tensor(
                    dst[oc][:, sl], rot[:], sin_sb[:, sl], OP.mult)
                nc.vector.tensor_tensor(
                    dst[oc][:, sl], dst[oc][:, sl], m[:], OP.add)
                yield

            def gen_v_unit(sb):
                ps = spp.tile([P, QSB], F32, tag="sp", name="ps")
                xsc, xo = sb // 4, (sb % 4) * P
                for dc in range(4):
                    nc.tensor.matmul(
                        ps[:], xTs[xsc][:, dc, xo:xo + P], wv[:, dc, :],
                        start=(dc == 0), stop=False)
                yield
                for dc in range(4, DC):
                    nc.tensor.matmul(
                        ps[:], xTs[xsc][:, dc, xo:xo + P], wv[:, dc, :],
                        start=False, stop=(dc == DC - 1))
                nc.scalar.copy(
                    V[sb][:, :, 0:DH],
                    ps[:].rearrange("p (h j) -> p h j", j=DH))
                yield

            wos = []

            def gen_op_sc(sc, jlo, jhi, dest_d, dma_eng, cast_engs):
                """One seq-block of an out-proj half: 8 oc units cast into a
                staging tile, then one 1 MB DMA with 8 KB/partition lines."""
                outs = outp.tile([P, 8, QSB], BF16, tag="outs", name="outs")
                for oc in range(8):
                    ps = spp.tile([P, QSB], F32, tag="sp", name="ps")
                    for jc in range(jlo, jhi):
                        nc.tensor.matmul(
                            ps[:], wos[oc][:, jc, :],
                            aT[jc][:, sc * QSB:(sc + 1) * QSB],
                            start=(jc == jlo), stop=(jc == jhi - 1))
                    eng = cast_engs[oc % len(cast_engs)]
                    eng_copy = (nc.scalar.copy if eng == "s"
                                else nc.vector.tensor_copy)
                    eng_copy(outs[:, oc, :], ps[:])
                    yield
                dma_eng.dma_start(dest_d[sc], outs[:])

            pending_norm = []

            def flush_norm():
                while pending_norm:
                    pending_norm.pop(0)()

            def gen_attn_dual(tq, qsb):
                """Attention for head pair (2tq, 2tq+1) over one q super-block.

                Per k-block: dual row-tiled QK (concurrent), one Exp over both
                heads, causal mask on the valid range only, sequential PV.
                """
                he, ho = 2 * tq, 2 * tq + 1
                qsl = slice(qsb * QSB, (qsb + 1) * QSB)
                blocks = plan[qsb]
                nblk = len(blocks)
                pv = pvp.tile([DH + 1, 2, QSB], F32, tag="pv", name="pv")

                def emit_pv(pt2, lo, kb, bi):
                    nc.tensor.matmul(
                        pv[:, 0, lo:QSB], V[kb][:, he, :], pt2[:, 0, lo:QSB],
                        start=(bi == 0), stop=(bi == nblk - 1))
                    nc.tensor.matmul(
                        pv[:, 1, lo:QSB], V[kb][:, ho, :], pt2[:, 1, lo:QSB],
                        start=(bi == 0), stop=(bi == nblk - 1))

                prev = None   # PV runs one step behind QK so a PV waiting on
                # exp/mask never blocks the next QK in the in-order PE queue
                for bi, (kb, msel) in enumerate(blocks):
                    lo = 0
                    if msel is not None and msel[0] == "const":
                        lo = KB * msel[1]
                    ksl = slice(kb * KB, (kb + 1) * KB)
                    qlo = slice(qsb * QSB + lo, (qsb + 1) * QSB)
                    st2 = stp.tile([P, 2, QSB], F32, tag="st", name="st2")
                    # dual row-tiled QK: base partitions 0 / 64 -> disjoint
                    # PE quadrant rows -> concurrent execution
                    nc.tensor.matmul(
                        st2[:, 0, lo:QSB], KTb[tq][0:DH, ksl],
                        QTb[tq][0:DH, qlo], start=True, stop=True)
                    nc.tensor.matmul(
                        st2[:, 1, lo:QSB], KTb[tq][DH:P, ksl],
                        QTb[tq][DH:P, qlo], start=True, stop=True)
                    if prev is not None:
                        emit_pv(*prev)
                    pt2 = ptpool.tile([P, 2, QSB], BF16, tag="pt")
                    nc.scalar.activation(
                        pt2[:, :, lo:QSB], st2[:, :, lo:QSB], AF.Exp,
                        scale=SCALE)
                    if msel is not None:
                        if msel[0] == "const":
                            r = msel[1]
                            for j in range(2):
                                nc.vector.tensor_tensor(
                                    pt2[:, j, lo:QSB], pt2[:, j, lo:QSB],
                                    mk[:, r, lo:QSB], OP.mult)
                        else:
                            mg = work.tile([P, QSB], F32, tag="ropem")
                            nc.sync.dma_start(mg[:], m01_d[msel[1], msel[2]])
                            mgb = ptpool.tile(
                                [P, 2, QSB], BF16, tag="pt", name="mgb")
                            nc.vector.tensor_copy(mgb[:, 0, :], mg[:])
                            for j in range(2):
                                nc.vector.tensor_tensor(
                                    pt2[:, j, :], pt2[:, j, :],
                                    mgb[:, 0, :], OP.mult)
                    prev = (pt2, lo, kb, bi)
                    if bi == 0:
                        # run the previous group's deferred normalization now,
                        # one block into this group, so its PE broadcast hides
                        # behind fresh QK work instead of stalling the stream
                        flush_norm()
                    yield
                emit_pv(*prev)

                # evacuate pv to SBUF so the single PSUM pv buffer recycles:
                # row sums (partition 64, banks 0/1) -> partitions 0/32 of
                # the rotating lrow tile; head dims -> aT (odd head crosses
                # quadrants 0->2; 64-row DVE ops support cross-quadrant writes)
                lrow2 = lrows[(tq * NQSB + qsb) % 2]
                with nc.allow_low_precision(reason="f32r rounding of l"):
                    nc.vector.tensor_copy(lrow2[0:1, :], pv[DH:DH + 1, 0, :])
                    nc.vector.tensor_copy(lrow2[32:33, :], pv[DH:DH + 1, 1, :])
                with nc.allow_low_precision(reason="bf16 aT evac"):
                    nc.vector.tensor_copy(aT[tq][0:DH, qsl], pv[0:DH, 0, :])
                    nc.vector.tensor_copy(aT[tq][DH:P, qsl], pv[0:DH, 1, :])

                def _norm(tq=tq, qsl=qsl, lrow2=lrow2):
                    # one PE matmul broadcasts l_e to partitions 0-63 and
                    # l_o to 64-127 (sel2 selector), 1/ via fast approx on
                    # the full 128-row tile, one in-place mult for both heads
                    bc = spp.tile([P, QSB], F32, tag="sp", name="bc")
                    nc.tensor.matmul(bc[:], sel2[:], lrow2[:],
                                     start=True, stop=True)
                    binv = work.tile([P, QSB], F32, tag="nrm2",
                                     name="binv", bufs=2)
                    nc.vector.reciprocal_approx_fast(binv[:], bc[:])
                    nc.vector.tensor_tensor(
                        aT[tq][:, qsl], aT[tq][:, qsl], binv[:], OP.mult)
                pending_norm.append(_norm)
                yield

            def drain(g):
                for _ in g:
                    pass

            def chain(gens):
                for g in gens:
                    yield from g

            def weave(agen, pgen, ratio):
                """Drain agen; after each yield, advance pgen by `ratio`."""
                acc = 0.0
                alive = True
                for _ in agen:
                    if not alive:
                        continue
                    acc += ratio
                    while acc >= 1.0:
                        if next(pgen, _SENT) is _SENT:
                            alive = False
                            break
                        acc -= 1.0
                for _ in pgen:
                    pass

            _SENT = object()

            # ---------------- interleaved emission ----------------
            # tile 0 projections + all of V up front (V feeds every round,
            # and trace order defines the dependency semantics); grouped by
            # x chunk so early units only wait on early DMA arrivals
            for sc in range(4):
                drain(gen_qk_unit(wqT_d, QTb, 0, sc, "q"))
                drain(gen_qk_unit(wkT_d, KTb, 0, sc, "k"))
                for sb in range(4 * sc, 4 * sc + 4):
                    drain(gen_v_unit(sb))

            # rounds: dual-head attention for tile t woven with tile t+1
            # projections / (round 3) the first half of the output projection,
            # so the PE always has ACT-free matmul work within a HAM window
            n_attn_yields = sum(len(row) for row in plan) + NQSB
            for t in range(4):
                if t == 1:
                    # prefetch all out-proj weights (needed from round 3 on)
                    for oc in range(8):
                        wo = wstream.tile([P, JC, P], BF16, tag="wo", bufs=8,
                                          name=f"wo{oc}")
                        nc.sync.dma_start(wo[:], woT_d[oc])
                        wos.append(wo)
                agen = chain([gen_attn_dual(t, qsb) for qsb in range(NQSB)])
                pgens = []
                if t < 3:
                    for sc in range(4):
                        pgens.append(
                            gen_qk_unit(wqT_d, QTb, t + 1, sc, "q", True))
                    for sc in range(4):
                        pgens.append(
                            gen_qk_unit(wkT_d, KTb, t + 1, sc, "k", True))
                    n_steps = 40
                else:
                    for sc in range(4):
                        pgens.append(
                            gen_op_sc(sc, 0, 2, outT_d, nc.sync, "v"))
                    n_steps = 32
                weave(agen, chain(pgens), n_steps / n_attn_yields)

            flush_norm()
            # second half of the output projection (aT[2], aT[3]); stores
            # alternate between the sync and gpsimd DMA queues, casts
            # alternate between ScalarE and VectorE (both idle in the tail)
            for sc in range(4):
                eng = nc.gpsimd if sc % 2 == 0 else nc.sync
                drain(gen_op_sc(sc, 2, JC, outB_d, eng, "sv"))

    nc.compile()
    return nc


def _plan_key(plan, mode):
    return (mode, tuple(tuple(row) for row in plan))


def _get_compiled(mask):
    plan, mode = _mask_plan(mask)
    key = _plan_key(plan, mode)
    if key not in _COMPILED:
        _COMPILED[key] = (_build(plan, mode), plan, mode)
    return _COMPILED[key]


# --------------------------------------------------------------- host driver

def _make_in_maps(x, Wq, Wk, Wv, Wo, mask, mode):
    cosT2, sinT2, psigT = _host_consts()
    consts = {"cosT": cosT2.astype(ml_dtypes.bfloat16),
              "sinT": sinT2.astype(ml_dtypes.bfloat16),
              "psgT": psigT.astype(ml_dtypes.bfloat16),
              "one64": np.ones((1, DH), np.float32),
              "sel2": _sel2_const()}
    if mode == "causal":
        m01 = np.zeros((4, P, QSB), np.float32)
        for r in range(4):
            for k in range(P):
                q0 = KB * r + k
                if q0 < QSB:
                    m01[r, k, q0:] = 1.0
        # [P, 4, QSB] pre-arranged for contiguous DMA
        consts["m01"] = np.ascontiguousarray(
            m01.transpose(1, 0, 2)).astype(ml_dtypes.bfloat16)
    elif mode == "generic":
        m = (np.asarray(mask).reshape(S, S) != 0)
        m01 = np.zeros((NQSB, NKB, P, QSB), np.float32)
        for qsb in range(NQSB):
            for kb in range(NKB):
                blk = m[qsb * QSB:(qsb + 1) * QSB, kb * KB:(kb + 1) * KB]
                m01[qsb, kb] = blk.T.astype(np.float32)
        consts["m01"] = m01

    def arr_qk(w):
        # [D, OG_rows] -> per-oc [P, DC, P]: wT[d, o] laid out [oc, p(o), dc, o']
        wT = w.T.astype(np.float32)                       # [D, OG]
        a = wT.reshape(DC, P, 4, P)          # [dc, p(d), oc, o']
        return np.ascontiguousarray(a.transpose(2, 1, 0, 3)).astype(
            ml_dtypes.bfloat16)

    in_maps = []
    for c in range(NCORES):
        b, g = c // HG, c % HG
        rows = slice(OG * g, OG * (g + 1))
        xT = x[b].T.astype(np.float32)                    # [D, S]
        xTa = np.ascontiguousarray(
            xT.reshape(DC, P, NQSB, QSB).transpose(2, 1, 0, 3)).astype(
                ml_dtypes.bfloat16)
        wq = arr_qk(Wq[rows, :])
        wk = arr_qk(Wk[rows, :])
        wvT = np.ascontiguousarray(
            Wv[rows, :].T.astype(np.float32).reshape(DC, P, OG)
            .transpose(1, 0, 2)).astype(ml_dtypes.bfloat16)
        woT = Wo[:, rows].T.astype(np.float32)            # [OG, D]
        woa = np.ascontiguousarray(
            woT.reshape(JC, P, 8, P).transpose(2, 1, 0, 3)
        ).astype(ml_dtypes.bfloat16)
        in_maps.append({
            "xT": xTa,
            "wqT": wq,
            "wkT": wk,
            "wvT": wvT,
            "woT": woa,
            **consts,
        })
    return in_maps


def run(x, Wq, Wk, Wv, Wo, mask, trace=False):
    nc, plan, mode = _get_compiled(mask)
    in_maps = _make_in_maps(x, Wq, Wk, Wv, Wo, mask, mode)
    res = bass_utils.run_bass_kernel_spmd(
        nc, in_maps, core_ids=list(range(NCORES)), trace=trace)
    out = np.empty((B, S, D), np.float32)
    for b in range(B):
        acc = (res.results[2 * b]["outT"].astype(np.float32)
               + res.results[2 * b]["outB"].astype(np.float32)
               + res.results[2 * b + 1]["outT"].astype(np.float32)
               + res.results[2 * b + 1]["outB"].astype(np.float32))
        # [NQSB, P, 8, QSB] = [sc, p, oc, q] -> [D=oc*128+p, S=sc*512+q] -> T
        out[b] = acc.transpose(2, 1, 0, 3).reshape(D, S).T
    return out, res


def kernel(x, Wq, Wk, Wv, Wo, mask):
    x = np.asarray(x, dtype=np.float32)
    Wq = np.asarray(Wq, dtype=np.float32)
    Wk = np.asarray(Wk, dtype=np.float32)
    Wv = np.asarray(Wv, dtype=np.float32)
    Wo = np.asarray(Wo, dtype=np.float32)
    out, _ = run(x, Wq, Wk, Wv, Wo, mask)
    return out


# revision 28
# speedup vs baseline: 1.1112x; 1.0312x over previous
"""Multi-head attention (RoPE, causal) Trainium2 Bass kernel, 8 NeuronCores.

Problem: x[4,2048,1024] -> MHA(16 heads, head_dim 64, RoPE, causal mask) -> [4,2048,1024]

Sharding (pure data/tensor parallel, no collectives):
  core c -> (batch b = c//2, head-group g = c%2); each head-group = 8 heads = 512 dims.
  Each core computes q/k/v projections for its (batch, head-group), RoPE, attention,
  and a partial output projection (columns of Wo for its head group).
  Host sums the two partial outputs per batch (512-dim contraction split).

Kernel layout tricks:
  - Projections computed in transposed [out_dim, seq] layout (QT/KT) so that
    QK^T blocks come out as S^T [k, q]: softmax reductions along the partition
    dim are avoided entirely via UNSAFE softmax (no row-max; inputs are bounded
    N(0,1)-ish data, logits stay << 88) and the row-sum is folded into the PV
    matmul by augmenting V with a ones column.  No on-chip transposes anywhere.
  - Dual-head attention: the two heads of a 128-row Q/K tile pair occupy
    partitions 0-63 / 64-127.  Their QK^T matmuls contract over only 64
    partitions each, so they are issued back-to-back: the PE row-tiling
    (tile_position auto-derived from base partitions 0 / 64) runs them
    CONCURRENTLY in disjoint quadrant rows -> ~2x effective QK throughput.
  - One Exp activation per k-block covers both heads ([128, 2, 512] PSUM
    pair-tile); causal diagonal blocks only exp/mask/PV the valid q-range
    (no memsets, narrower matmuls).
  - PV accumulates into a [65, 2, 512] PSUM tile (ones column -> row-sums at
    partition 64); at group end the tile is evacuated to SBUF by DVE casts
    (odd head cast crosses quadrants 0->2, HW-supported for 64-row ops) so
    the single PSUM pv buffer recycles immediately; normalization (recip +
    PE broadcast of 1/l + one in-place multiply for both heads) is deferred
    one group to hide behind fresh QK work.
  - Projections / out-proj in bf16; attention QK'/PV in bf16.
  - Output written as [4, 1024, 512] seq-blocks so every store is one fully
    contiguous 128 KB DMA; tail out-proj stores alternate between the sync
    and gpsimd DMA queues to double drain bandwidth.
"""

import numpy as np
import ml_dtypes

import concourse.bass as bass
import concourse.tile as tile
from concourse import bacc, mybir
from concourse import bass_utils

B, S, D, H, DH = 4, 2048, 1024, 16, 64
NCORES = 8
HG = 2              # head groups (tensor parallel)
HPG = H // HG       # heads per group = 8
OG = HPG * DH       # group output dims = 512
SCALE = DH ** -0.5
P = 128
QSB = 512           # q super-block width
NQSB = S // QSB     # 4
KB = 128            # k block
NKB = S // KB       # 16
DC = D // P         # 8 d-chunks
JC = OG // P        # 4 j-chunks (out-proj contraction)

F32 = mybir.dt.float32
F32R = mybir.dt.float32r
BF16 = mybir.dt.bfloat16

_COMPILED = {}


# ---------------------------------------------------------------- host tables

def _rope_tables():
    inv_freq = 1.0 / (10000.0 ** (np.arange(0, DH, 2, dtype=np.float32) / DH))
    t = np.arange(S, dtype=np.float32)
    freqs = np.outer(t, inv_freq).astype(np.float32)      # [S, 32]
    emb = np.concatenate([freqs, freqs], -1)              # [S, 64]
    return np.cos(emb), np.sin(emb)


def _sel2_const():
    # broadcast selector: row 0 -> out partitions 0-63 (l_e), row 32 -> out
    # partitions 64-127 (l_o)
    sel = np.zeros((DH, P), np.float32)
    sel[0, 0:DH] = 1.0
    sel[32, DH:P] = 1.0
    return sel


def _host_consts():
    cos, sin = _rope_tables()                             # [S, 64]
    cosT2 = np.ascontiguousarray(
        np.concatenate([cos.T, cos.T], axis=0), dtype=np.float32)   # [128, S]
    sinT2 = np.ascontiguousarray(
        np.concatenate([sin.T, sin.T], axis=0), dtype=np.float32)
    # signed permutation: rot(x)[i] = -x[i+32] (j<32) else x[i-32], per 64-row head
    psig = np.zeros((P, P), np.float32)
    for i in range(P):
        j = i % DH
        base = (i // DH) * DH
        if j < 32:
            psig[i, base + j + 32] = -1.0
        else:
            psig[i, base + j - 32] = 1.0
    psigT = np.ascontiguousarray(psig.T)
    return cosT2, sinT2, psigT


def _mask_plan(mask):
    """Classify the [S, S] mask into a per-qsb block plan.

    plan[qsb] = list of (kb, msel); msel is None (no mask), ("const", r) for
    the 4 shared causal diagonal tiles, or ("dram", qsb, kb) for generic
    per-block mask tiles.
    """
    m = np.asarray(mask).reshape(S, S) != 0        # [q, k] True = attend
    causal = np.array_equal(m, np.tril(np.ones((S, S), bool)))
    if causal:
        plan = []
        for qsb in range(NQSB):
            row = []
            for kb in range(4 * qsb + 4):
                r = kb - 4 * qsb
                row.append((kb, None if r < 0 else ("const", r)))
            plan.append(row)
        return plan, "causal"
    if m.all():
        return [[(kb, None) for kb in range(NKB)] for _ in range(NQSB)], "full"
    plan = []
    for qsb in range(NQSB):
        row = []
        for kb in range(NKB):
            blk = m[qsb * QSB:(qsb + 1) * QSB, kb * KB:(kb + 1) * KB]  # [q, k]
            if not blk.any():
                continue          # fully masked block contributes nothing
            row.append((kb, None if blk.all() else ("dram", qsb, kb)))
        plan.append(row)
    return plan, "generic"


# ------------------------------------------------------------------- builder

def _build(plan, mode):
    nc = bacc.Bacc("TRN2", target_bir_lowering=False, debug=False, num_devices=1)
    AF = mybir.ActivationFunctionType
    OP = mybir.AluOpType

    xT_d = nc.dram_tensor("xT", [NQSB, P, DC, QSB], BF16,
                          kind="ExternalInput").ap()
    wqT_d = nc.dram_tensor("wqT", [4, P, DC, P], BF16,
                           kind="ExternalInput").ap()
    wkT_d = nc.dram_tensor("wkT", [4, P, DC, P], BF16,
                           kind="ExternalInput").ap()
    wvT_d = nc.dram_tensor("wvT", [P, DC, OG], BF16, kind="ExternalInput").ap()
    woT_d = nc.dram_tensor("woT", [8, P, JC, P], BF16,
                           kind="ExternalInput").ap()
    cos_d = nc.dram_tensor("cosT", [P, S], BF16, kind="ExternalInput").ap()
    sin_d = nc.dram_tensor("sinT", [P, S], BF16, kind="ExternalInput").ap()
    psg_d = nc.dram_tensor("psgT", [P, P], BF16, kind="ExternalInput").ap()
    if mode == "causal":
        m01_d = nc.dram_tensor("m01", [P, 4, QSB], BF16, kind="ExternalInput").ap()
    elif mode == "generic":
        m01_d = nc.dram_tensor("m01", [NQSB, NKB, P, QSB], F32,
                               kind="ExternalInput").ap()
    else:
        m01_d = None
    one64_d = nc.dram_tensor("one64", [1, DH], F32R, kind="ExternalInput").ap()
    sel2_d = nc.dram_tensor("sel2", [DH, P], F32R, kind="ExternalInput").ap()
    # [sc, p, oc, q] blocked outputs: one 1 MB DMA per seq-block with fully
    # contiguous 8 KB per-partition lines (host re-assembles to [D, S])
    outT_d = nc.dram_tensor("outT", [NQSB, P, 8, QSB], BF16,
                            kind="ExternalOutput").ap()
    outB_d = nc.dram_tensor("outB", [NQSB, P, 8, QSB], BF16,
                            kind="ExternalOutput").ap()

    with tile.TileContext(nc) as tc:
        from contextlib import ExitStack
        with ExitStack() as ctx:
            persist = ctx.enter_context(tc.tile_pool(name="persist", bufs=1))
            wstream = ctx.enter_context(tc.tile_pool(name="wstream", bufs=2))
            work = ctx.enter_context(tc.tile_pool(name="work", bufs=2))
            prepool = ctx.enter_context(tc.tile_pool(name="prepool", bufs=2))
            ptpool = ctx.enter_context(tc.tile_pool(name="ptpool", bufs=4))
            outp = ctx.enter_context(tc.tile_pool(name="outp", bufs=2))
            # PSUM budget (8 banks): stp 2x[128,2,512] = 4, spp 2x[128,512]
            # = 2, pvp 1x[65,2,512] = 2
            stp = ctx.enter_context(
                tc.tile_pool(name="stp", bufs=2, space="PSUM"))
            spp = ctx.enter_context(
                tc.tile_pool(name="spp", bufs=2, space="PSUM"))
            pvp = ctx.enter_context(
                tc.tile_pool(name="pvp", bufs=1, space="PSUM"))

            # bf16 post-rope Q/K and bf16 V (with ones column) live all-kernel
            QTb = [persist.tile([P, S], BF16, tag=f"qt{t}", name=f"qtb{t}")
                   for t in range(4)]
            KTb = [persist.tile([P, S], BF16, tag=f"kt{t}", name=f"ktb{t}")
                   for t in range(4)]
            V = [persist.tile([P, HPG, DH + 1], BF16, tag=f"v{sb}",
                              name=f"v{sb}") for sb in range(NKB)]
            for sb in range(NKB):
                nc.vector.memset(V[sb][:, :, DH:DH + 1], 1.0)

            xTs = [persist.tile([P, DC, QSB], BF16, tag=f"xt{sc}",
                                 name=f"xt{sc}") for sc in range(4)]
            # demand-ordered startup loads: the first projection matmuls wait
            # only on wq0 + a 256 KB x chunk (dependency tracking is
            # DMA-range-based, so finer chunks unblock the PE sooner)
            wqk_live = {}
            wq0 = wstream.tile([P, DC, P], BF16, tag="wqk", name="wq0")
            nc.sync.dma_start(wq0[:], wqT_d[0])
            wqk_live["q"] = wq0
            nc.sync.dma_start(xTs[0][:, 0:2, :], xT_d[0][:, 0:2, :])
            nc.sync.dma_start(xTs[0][:, 2:4, :], xT_d[0][:, 2:4, :])
            wk0 = wstream.tile([P, DC, P], BF16, tag="wqk", name="wk0")
            nc.sync.dma_start(wk0[:], wkT_d[0])
            wqk_live["k"] = wk0
            nc.sync.dma_start(xTs[0][:, 4:6, :], xT_d[0][:, 4:6, :])
            nc.sync.dma_start(xTs[0][:, 6:DC, :], xT_d[0][:, 6:DC, :])
            psg_sb = persist.tile([P, P], BF16, tag="psg")
            nc.sync.dma_start(psg_sb[:], psg_d)
            wv = persist.tile([P, DC, OG], BF16, tag="wv")
            nc.gpsimd.dma_start(wv[:], wvT_d)
            cos_sb = persist.tile([P, S], BF16, tag="cos")
            sin_sb = persist.tile([P, S], BF16, tag="sin")
            nc.gpsimd.dma_start(cos_sb[:], cos_d)
            nc.gpsimd.dma_start(sin_sb[:], sin_d)
            for sc in range(1, 4):
                nc.sync.dma_start(xTs[sc][:], xT_d[sc])
            aT = [persist.tile([P, S], BF16, tag=f"at{t}", name=f"at{t}")
                  for t in range(4)]
            ones64 = persist.tile([1, DH], F32R, tag="ones64")
            nc.gpsimd.dma_start(ones64[:], one64_d)
            sel2 = persist.tile([DH, P], F32R, tag="sel2")
            nc.gpsimd.dma_start(sel2[:], sel2_d)
            # l staging rows: l_e at partition 0, l_o at partition 32 (DVE
            # partition writes must be 32-aligned); memset once so the unused
            # rows the broadcast matmul reads are never NaN
            lrows = [persist.tile([DH, QSB], F32R, tag=f"lr{i}",
                                  name=f"lrows{i}") for i in range(2)]
            for i in range(2):
                nc.vector.memset(lrows[i][:].bitcast(F32), 1.0)
            if mode == "causal":
                mk = persist.tile([P, 4, QSB], BF16, tag="m01")
                nc.gpsimd.dma_start(mk[:], m01_d)

            # ---------------- emitters (generators) ----------------
            # yield points let attention steps and projection halves weave at
            # ~1 us granularity so the PE never sees an ACT-bound stretch

            def gen_qk_unit(w_d, dst, oc, sc, who, woven=False):
                """One [128, 512] chunk of a Q/K projection + RoPE.

                Fine-grained yields (2 matmuls per step) keep PE insertions
                between attention steps small so the Exp stream never starves.
                """
                if sc == 0 and oc > 0:
                    w_oc = wstream.tile([P, DC, P], BF16, tag="wqk",
                                        name=f"w{who}{oc}")
                    nc.sync.dma_start(w_oc[:], w_d[oc])
                    wqk_live[who] = w_oc
                w_oc = wqk_live[who]
                sl = slice(sc * QSB, (sc + 1) * QSB)
                ps = spp.tile([P, QSB], F32, tag="sp", name="ps")
                for dc in range(DC):
                    nc.tensor.matmul(
                        ps[:], w_oc[:, dc, :], xTs[sc][:, dc, :],
                        start=(dc == 0), stop=(dc == DC - 1))
                    if dc % 2 == 1 and dc < DC - 1:
                        yield
                pre = prepool.tile([P, QSB], BF16, tag="pre")
                # pre copy off ScalarE during attention rounds (Exp is the
                # critical engine there); ScalarE when woven upfront
                if woven:
                    nc.vector.tensor_copy(pre[:], ps[:])
                else:
                    nc.scalar.copy(pre[:], ps[:])
                yield
                rot = spp.tile([P, QSB], F32, tag="sp", name="rot")
                nc.tensor.matmul(rot[:], psg_sb[:], pre[:],
                                 start=True, stop=True)
                m = work.tile([P, QSB], BF16, tag="ropem")
                nc.vector.tensor_tensor(m[:], pre[:], cos_sb[:, sl], OP.mult)
                nc.vector.tensor_tensor(
                    dst[oc][:, sl], rot[:], sin_sb[:, sl], OP.mult)
                nc.vector.tensor_tensor(
                    dst[oc][:, sl], dst[oc][:, sl], m[:], OP.add)
                yield

            def gen_v_unit(sb, woven=False):
                ps = spp.tile([P, QSB], F32, tag="sp", name="ps")
                xsc, xo = sb // 4, (sb % 4) * P
                for dc in range(4):
                    nc.tensor.matmul(
                        ps[:], xTs[xsc][:, dc, xo:xo + P], wv[:, dc, :],
                        start=(dc == 0), stop=False)
                yield
                for dc in range(4, DC):
                    nc.tensor.matmul(
                        ps[:], xTs[xsc][:, dc, xo:xo + P], wv[:, dc, :],
                        start=False, stop=(dc == DC - 1))
                copy_eng = nc.vector.tensor_copy if woven else nc.scalar.copy
                copy_eng(
                    V[sb][:, :, 0:DH],
                    ps[:].rearrange("p (h j) -> p h j", j=DH))
                yield

            wos = []

            def gen_op_sc(sc, jlo, jhi, dest_d, dma_eng, cast_engs):
                """One seq-block of an out-proj half: 8 oc units cast into a
                staging tile, then one 1 MB DMA with 8 KB/partition lines."""
                outs = outp.tile([P, 8, QSB], BF16, tag="outs", name="outs")
                for oc in range(8):
                    ps = spp.tile([P, QSB], F32, tag="sp", name="ps")
                    for jc in range(jlo, jhi):
                        nc.tensor.matmul(
                            ps[:], wos[oc][:, jc, :],
                            aT[jc][:, sc * QSB:(sc + 1) * QSB],
                            start=(jc == jlo), stop=(jc == jhi - 1))
                    eng = cast_engs[oc % len(cast_engs)]
                    eng_copy = (nc.scalar.copy if eng == "s"
                                else nc.vector.tensor_copy)
                    eng_copy(outs[:, oc, :], ps[:])
                    yield
                dma_eng.dma_start(dest_d[sc], outs[:])

            pending_norm = []

            def flush_norm():
                while pending_norm:
                    pending_norm.pop(0)()

            def gen_attn_dual(tq, qsb):
                """Attention for head pair (2tq, 2tq+1) over one q super-block.

                Per k-block: dual row-tiled QK (concurrent), one Exp over both
                heads, causal mask on the valid range only, sequential PV.
                """
                he, ho = 2 * tq, 2 * tq + 1
                qsl = slice(qsb * QSB, (qsb + 1) * QSB)
                blocks = plan[qsb]
                nblk = len(blocks)
                pv = pvp.tile([DH + 1, 2, QSB], F32, tag="pv", name="pv")

                def emit_pv(pt2, lo, kb, bi):
                    nc.tensor.matmul(
                        pv[:, 0, lo:QSB], V[kb][:, he, :], pt2[:, 0, lo:QSB],
                        start=(bi == 0), stop=(bi == nblk - 1))
                    nc.tensor.matmul(
                        pv[:, 1, lo:QSB], V[kb][:, ho, :], pt2[:, 1, lo:QSB],
                        start=(bi == 0), stop=(bi == nblk - 1))

                prevs = []   # PV runs TWO steps behind QK: the in-order PE
                # queue then never blocks on a PV whose exp/mask chain is
                # still in flight, so the Exp stream stays saturated
                for bi, (kb, msel) in enumerate(blocks):
                    lo = 0
                    if msel is not None and msel[0] == "const":
                        lo = KB * msel[1]
                    ksl = slice(kb * KB, (kb + 1) * KB)
                    qlo = slice(qsb * QSB + lo, (qsb + 1) * QSB)
                    st2 = stp.tile([P, 2, QSB], F32, tag="st", name="st2")
                    # dual row-tiled QK: base partitions 0 / 64 -> disjoint
                    # PE quadrant rows -> concurrent execution
                    nc.tensor.matmul(
                        st2[:, 0, lo:QSB], KTb[tq][0:DH, ksl],
                        QTb[tq][0:DH, qlo], start=True, stop=True)
                    nc.tensor.matmul(
                        st2[:, 1, lo:QSB], KTb[tq][DH:P, ksl],
                        QTb[tq][DH:P, qlo], start=True, stop=True)
                    if len(prevs) >= 2:
                        emit_pv(*prevs.pop(0))
                    pt2 = ptpool.tile([P, 2, QSB], BF16, tag="pt")
                    nc.scalar.activation(
                        pt2[:, :, lo:QSB], st2[:, :, lo:QSB], AF.Exp,
                        scale=SCALE)
                    if msel is not None:
                        if msel[0] == "const":
                            r = msel[1]
                            for j in range(2):
                                nc.vector.tensor_tensor(
                                    pt2[:, j, lo:QSB], pt2[:, j, lo:QSB],
                                    mk[:, r, lo:QSB], OP.mult)
                        else:
                            mg = work.tile([P, QSB], F32, tag="ropem")
                            nc.sync.dma_start(mg[:], m01_d[msel[1], msel[2]])
                            mgb = ptpool.tile(
                                [P, 2, QSB], BF16, tag="pt", name="mgb")
                            nc.vector.tensor_copy(mgb[:, 0, :], mg[:])
                            for j in range(2):
                                nc.vector.tensor_tensor(
                                    pt2[:, j, :], pt2[:, j, :],
                                    mgb[:, 0, :], OP.mult)
                    prevs.append((pt2, lo, kb, bi))
                    if bi == 0:
                        # run the previous group's deferred normalization now,
                        # one block into this group, so its PE broadcast hides
                        # behind fresh QK work instead of stalling the stream
                        flush_norm()
                    yield
                for pr in prevs:
                    emit_pv(*pr)

                # evacuate pv to SBUF so the single PSUM pv buffer recycles:
                # row sums (partition 64, banks 0/1) -> partitions 0/32 of
                # the rotating lrow tile; head dims -> aT (odd head crosses
                # quadrants 0->2; 64-row DVE ops support cross-quadrant writes)
                lrow2 = lrows[(tq * NQSB + qsb) % 2]
                with nc.allow_low_precision(reason="f32r rounding of l"):
                    nc.vector.tensor_copy(lrow2[0:1, :], pv[DH:DH + 1, 0, :])
                    nc.vector.tensor_copy(lrow2[32:33, :], pv[DH:DH + 1, 1, :])
                with nc.allow_low_precision(reason="bf16 aT evac"):
                    nc.vector.tensor_copy(aT[tq][0:DH, qsl], pv[0:DH, 0, :])
                    nc.vector.tensor_copy(aT[tq][DH:P, qsl], pv[0:DH, 1, :])

                def _norm(tq=tq, qsl=qsl, lrow2=lrow2):
                    # one PE matmul broadcasts l_e to partitions 0-63 and
                    # l_o to 64-127 (sel2 selector), 1/ via fast approx on
                    # the full 128-row tile, one in-place mult for both heads
                    bc = spp.tile([P, QSB], F32, tag="sp", name="bc")
                    nc.tensor.matmul(bc[:], sel2[:], lrow2[:],
                                     start=True, stop=True)
                    binv = work.tile([P, QSB], F32, tag="nrm2",
                                     name="binv", bufs=2)
                    nc.vector.reciprocal_approx_fast(binv[:], bc[:])
                    nc.vector.tensor_tensor(
                        aT[tq][:, qsl], aT[tq][:, qsl], binv[:], OP.mult)
                pending_norm.append(_norm)
                yield

            def drain(g):
                for _ in g:
                    pass

            def chain(gens):
                for g in gens:
                    yield from g

            def weave(agen, pgen, ratio):
                """Drain agen; after each yield, advance pgen by `ratio`."""
                acc = 0.0
                alive = True
                for _ in agen:
                    if not alive:
                        continue
                    acc += ratio
                    while acc >= 1.0:
                        if next(pgen, _SENT) is _SENT:
                            alive = False
                            break
                        acc -= 1.0
                for _ in pgen:
                    pass

            _SENT = object()

            # ---------------- interleaved emission ----------------
            # minimal upfront: only what attention groups (0,0)/(0,1) need
            # (tile-0 sc0/sc1 projections + V0-7); everything else weaves
            # into the rounds, demand-ordered so each attention group's
            # prerequisites land ~1 group ahead of use
            for sc in range(2):
                drain(gen_qk_unit(wqT_d, QTb, 0, sc, "q"))
                drain(gen_qk_unit(wkT_d, KTb, 0, sc, "k"))
                for sb in range(4 * sc, 4 * sc + 4):
                    drain(gen_v_unit(sb))

            # rounds: dual-head attention for tile t woven with the rest of
            # tile t's projections (sc2/3), tile t+1's early ones (sc0/1),
            # and (round 3) the first out-proj half, so the PE always has
            # ACT-free matmul work and ScalarE exp never idles
            n_attn_yields = sum(len(row) for row in plan) + NQSB
            for t in range(4):
                if t == 1:
                    # prefetch all out-proj weights (needed from round 3 on)
                    for oc in range(8):
                        wo = wstream.tile([P, JC, P], BF16, tag="wo", bufs=8,
                                          name=f"wo{oc}")
                        nc.sync.dma_start(wo[:], woT_d[oc])
                        wos.append(wo)
                agen = chain([gen_attn_dual(t, qsb) for qsb in range(NQSB)])
                pgens = []
                n_steps = 0
                # this tile's sc2/sc3 projections (needed by groups 2/3)
                for sc in (2, 3):
                    pgens.append(gen_qk_unit(wqT_d, QTb, t, sc, "q", True))
                    pgens.append(gen_qk_unit(wkT_d, KTb, t, sc, "k", True))
                    n_steps += 10
                    if t == 0:
                        for sb in range(4 * sc, 4 * sc + 4):
                            pgens.append(gen_v_unit(sb, True))
                            n_steps += 2
                if t < 3:
                    # next tile's sc0/sc1 projections (needed next round)
                    for sc in (0, 1):
                        pgens.append(
                            gen_qk_unit(wqT_d, QTb, t + 1, sc, "q", True))
                        pgens.append(
                            gen_qk_unit(wkT_d, KTb, t + 1, sc, "k", True))
                        n_steps += 10
                else:
                    for sc in range(4):
                        pgens.append(
                            gen_op_sc(sc, 0, 2, outT_d, nc.sync, "v"))
                        n_steps += 8
                weave(agen, chain(pgens), n_steps / n_attn_yields)

            flush_norm()
            # second half of the output projection (aT[2], aT[3]); stores
            # alternate between the sync and gpsimd DMA queues, casts
            # alternate between ScalarE and VectorE (both idle in the tail)
            for sc in range(4):
                eng = nc.gpsimd if sc % 2 == 0 else nc.sync
                drain(gen_op_sc(sc, 2, JC, outB_d, eng, "sv"))

    nc.compile()
    return nc


def _plan_key(plan, mode):
    return (mode, tuple(tuple(row) for row in plan))


def _get_compiled(mask):
    plan, mode = _mask_plan(mask)
    key = _plan_key(plan, mode)
    if key not in _COMPILED:
        _COMPILED[key] = (_build(plan, mode), plan, mode)
    return _COMPILED[key]


# --------------------------------------------------------------- host driver

def _make_in_maps(x, Wq, Wk, Wv, Wo, mask, mode):
    cosT2, sinT2, psigT = _host_consts()
    consts = {"cosT": cosT2.astype(ml_dtypes.bfloat16),
              "sinT": sinT2.astype(ml_dtypes.bfloat16),
              "psgT": psigT.astype(ml_dtypes.bfloat16),
              "one64": np.ones((1, DH), np.float32),
              "sel2": _sel2_const()}
    if mode == "causal":
        m01 = np.zeros((4, P, QSB), np.float32)
        for r in range(4):
            for k in range(P):
                q0 = KB * r + k
                if q0 < QSB:
                    m01[r, k, q0:] = 1.0
        # [P, 4, QSB] pre-arranged for contiguous DMA
        consts["m01"] = np.ascontiguousarray(
            m01.transpose(1, 0, 2)).astype(ml_dtypes.bfloat16)
    elif mode == "generic":
        m = (np.asarray(mask).reshape(S, S) != 0)
        m01 = np.zeros((NQSB, NKB, P, QSB), np.float32)
        for qsb in range(NQSB):
            for kb in range(NKB):
                blk = m[qsb * QSB:(qsb + 1) * QSB, kb * KB:(kb + 1) * KB]
                m01[qsb, kb] = blk.T.astype(np.float32)
        consts["m01"] = m01

    def arr_qk(w):
        # [D, OG_rows] -> per-oc [P, DC, P]: wT[d, o] laid out [oc, p(o), dc, o']
        wT = w.T.astype(np.float32)                       # [D, OG]
        a = wT.reshape(DC, P, 4, P)          # [dc, p(d), oc, o']
        return np.ascontiguousarray(a.transpose(2, 1, 0, 3)).astype(
            ml_dtypes.bfloat16)

    in_maps = []
    for c in range(NCORES):
        b, g = c // HG, c % HG
        rows = slice(OG * g, OG * (g + 1))
        xT = x[b].T.astype(np.float32)                    # [D, S]
        xTa = np.ascontiguousarray(
            xT.reshape(DC, P, NQSB, QSB).transpose(2, 1, 0, 3)).astype(
                ml_dtypes.bfloat16)
        wq = arr_qk(Wq[rows, :])
        wk = arr_qk(Wk[rows, :])
        wvT = np.ascontiguousarray(
            Wv[rows, :].T.astype(np.float32).reshape(DC, P, OG)
            .transpose(1, 0, 2)).astype(ml_dtypes.bfloat16)
        woT = Wo[:, rows].T.astype(np.float32)            # [OG, D]
        woa = np.ascontiguousarray(
            woT.reshape(JC, P, 8, P).transpose(2, 1, 0, 3)
        ).astype(ml_dtypes.bfloat16)
        in_maps.append({
            "xT": xTa,
            "wqT": wq,
            "wkT": wk,
            "wvT": wvT,
            "woT": woa,
            **consts,
        })
    return in_maps


def run(x, Wq, Wk, Wv, Wo, mask, trace=False):
    nc, plan, mode = _get_compiled(mask)
    in_maps = _make_in_maps(x, Wq, Wk, Wv, Wo, mask, mode)
    res = bass_utils.run_bass_kernel_spmd(
        nc, in_maps, core_ids=list(range(NCORES)), trace=trace)
    out = np.empty((B, S, D), np.float32)
    for b in range(B):
        acc = (res.results[2 * b]["outT"].astype(np.float32)
               + res.results[2 * b]["outB"].astype(np.float32)
               + res.results[2 * b + 1]["outT"].astype(np.float32)
               + res.results[2 * b + 1]["outB"].astype(np.float32))
        # [NQSB, P, 8, QSB] = [sc, p, oc, q] -> [D=oc*128+p, S=sc*512+q] -> T
        out[b] = acc.transpose(2, 1, 0, 3).reshape(D, S).T
    return out, res


def kernel(x, Wq, Wk, Wv, Wo, mask):
    x = np.asarray(x, dtype=np.float32)
    Wq = np.asarray(Wq, dtype=np.float32)
    Wk = np.asarray(Wk, dtype=np.float32)
    Wv = np.asarray(Wv, dtype=np.float32)
    Wo = np.asarray(Wo, dtype=np.float32)
    out, _ = run(x, Wq, Wk, Wv, Wo, mask)
    return out
